# revision 10
# baseline (speedup 1.0000x reference)
"""Bloom attention (separated QKV) — 8-core TRN2 Bass kernel.

Distribution: tensor-parallel over heads (2 heads/core). Each core:
  1. QKV projections for its 256-row slice of Wq/Wk/Wv (q^T,k^T in [d,s]
     layout, v in [s,d] layout, all bf16 in SBUF, fp32 accumulate).
  2. Attention for its (b, head) pairs with transposed scores
     St[k,q] = k @ q^T, exp via ScalarE (alibi as per-partition bias,
     1/sqrt(d) as scale), softmax denominator via ones-matmul,
     ctx^T = v^T @ P accumulated in PSUM, normalized by 1/den.
  3. AllGather of ctx^T slices (bf16) -> full ctx^T [H, B*S].
  4. Output projection for its 256-column slice of Wd + bias + residual.
Host side: transpose/slice/cast weights + hs (layout prep only),
concatenate the 8 output column-slices.
"""
import numpy as np
import ml_dtypes

import concourse.bass as bass
import concourse.bacc as bacc
import concourse.mybir as mybir
import concourse.tile as tile
import concourse.bass_utils as bass_utils

BF16 = ml_dtypes.bfloat16
N_CORES = 8
B, S, H = 2, 2048, 2048
NH, HD = 16, 128
HPC = NH // N_CORES          # heads per core
CI = HPC * HD                # per-core slice of H (256)
BS = B * S                   # 4096
INV_NORM = 1.0 / float(np.sqrt(HD))

JT = H // 128                # 16 contraction tiles for projections
SS_CHUNK = 512               # seq chunk for projections / scores free dim
N_CHUNKS = BS // SS_CHUNK    # 8
QC_PER_B = S // SS_CHUNK     # 4 query chunks per batch
KT = S // 128                # 16 key tiles per batch
IT = H // 128                # 16 contraction tiles for dense
SS_BLOCK = 8                 # ss tiles (128) per dense block
N_BLOCKS = BS // (SS_BLOCK * 128)   # 4

F32 = mybir.dt.float32
BF = mybir.dt.bfloat16

DEBUG_OUTPUTS = False


def _build():
    nc = bacc.Bacc("TRN2", target_bir_lowering=False, debug=False,
                   num_devices=N_CORES)

    hsT = nc.dram_tensor("hsT", [H, BS], BF, kind="ExternalInput").ap()
    wqT = nc.dram_tensor("wqT", [H, CI], BF, kind="ExternalInput").ap()
    wkT = nc.dram_tensor("wkT", [H, CI], BF, kind="ExternalInput").ap()
    wvT = nc.dram_tensor("wvT", [H, CI], BF, kind="ExternalInput").ap()
    wdT = nc.dram_tensor("wdT", [H, CI], BF, kind="ExternalInput").ap()
    bq = nc.dram_tensor("bq", [CI, 1], F32, kind="ExternalInput").ap()
    bk = nc.dram_tensor("bk", [CI, 1], F32, kind="ExternalInput").ap()
    bv = nc.dram_tensor("bv", [1, CI], BF, kind="ExternalInput").ap()
    bd = nc.dram_tensor("bd", [1, CI], BF, kind="ExternalInput").ap()
    alibi = nc.dram_tensor("alibi", [B * HPC, S], F32, kind="ExternalInput").ap()
    resid = nc.dram_tensor("resid", [BS, CI], F32, kind="ExternalInput").ap()
    out = nc.dram_tensor("out", [BS, CI], F32, kind="ExternalOutput").ap()

    ctxT_local = nc.dram_tensor("ctxT_local", [CI, BS], BF,
                                kind="Internal").ap()
    ctxT_full = nc.dram_tensor("ctxT_full", [H, BS], BF, kind="Internal",
                               addr_space="Shared").ap()
    if DEBUG_OUTPUTS:
        qT_dbg = nc.dram_tensor("qT_dbg", [128, HPC * BS], BF,
                                kind="ExternalOutput").ap()
        kT_dbg = nc.dram_tensor("kT_dbg", [128, HPC * BS], BF,
                                kind="ExternalOutput").ap()
        v_dbg = nc.dram_tensor("v_dbg", [128, (BS // 128) * CI], BF,
                               kind="ExternalOutput").ap()
        ctxT_dbg = nc.dram_tensor("ctxT_dbg", [H, BS], BF,
                                  kind="ExternalOutput").ap()

    with tile.TileContext(nc) as tc:
        with (
            tc.tile_pool(name="const", bufs=1) as constp,
            tc.tile_pool(name="qkv", bufs=1) as qkvp,
        ):
            # ---- phase 0: constants ----
            wq_sb = constp.tile([128, JT, CI], BF)
            wk_sb = constp.tile([128, JT, CI], BF)
            wv_sb = constp.tile([128, JT, CI], BF)
            wd_sb = constp.tile([128, IT, CI], BF)
            for w_sb, w_dr in ((wq_sb, wqT), (wk_sb, wkT), (wv_sb, wvT),
                               (wd_sb, wdT)):
                nc.sync.dma_start(
                    w_sb[:], w_dr.rearrange("(jt p) i -> p jt i", p=128))
            bq_sb = constp.tile([128, HPC], F32)
            bk_sb = constp.tile([128, HPC], F32)
            for b_sb, b_dr in ((bq_sb, bq), (bk_sb, bk)):
                for hi in range(HPC):
                    nc.sync.dma_start(b_sb[:, hi:hi + 1],
                                      b_dr[hi * 128:(hi + 1) * 128, :])
            bv_sb = constp.tile([1, CI], BF)
            bd_sb = constp.tile([1, CI], BF)
            nc.sync.dma_start(bv_sb[:], bv[:])
            nc.sync.dma_start(bd_sb[:], bd[:])
            alibi_sb = constp.tile([128, B * HPC, KT], F32)
            nc.sync.dma_start(
                alibi_sb[:], alibi.rearrange("r (kt p) -> p r kt", p=128))
            ones_col_bf = constp.tile([128, 1], BF)    # den lhsT (K=128, M=1)
            ones_row_bf = constp.tile([1, 128], BF)    # bias lhsT (K=1, M=128)
            ones_row_f32 = constp.tile([1, 128], F32)  # bcast lhsT (K=1, M=128)
            nc.vector.memset(ones_col_bf[:], 1.0)
            nc.vector.memset(ones_row_bf[:], 1.0)
            nc.vector.memset(ones_row_f32[:], 1.0)

            # persistent per-core activations
            qT_sb = qkvp.tile([128, HPC, BS], BF)      # [d, hi, ss]
            kT_sb = qkvp.tile([128, HPC, BS], BF)
            v_sb = qkvp.tile([128, BS // 128, CI], BF)  # [ss%128, ss//128, i]

            # ---- phase 1: QKV projections ----
            hsT_r = hsT.rearrange("(jt p) s -> p jt s", p=128)
            with (
                tc.tile_pool(name="hsb", bufs=3) as hsp,
                tc.tile_pool(name="p1psum", bufs=4,
                             space=bass.MemorySpace.PSUM) as p1p,
            ):
                for ch in range(N_CHUNKS):
                    s0 = ch * SS_CHUNK
                    hsb = hsp.tile([128, JT, SS_CHUNK], BF, name="hsb")
                    nc.sync.dma_start(hsb[:], hsT_r[:, :, s0:s0 + SS_CHUNK])
                    for w_sb, b_col, o_sb, scale in (
                        (wq_sb, bq_sb, qT_sb, INV_NORM),
                        (wk_sb, bk_sb, kT_sb, 1.0),
                    ):
                        for hi in range(HPC):
                            ps = p1p.tile([128, SS_CHUNK], F32, name="ps_qk")
                            for jt in range(JT):
                                nc.tensor.matmul(
                                    ps[:],
                                    w_sb[:, jt, hi * 128:(hi + 1) * 128],
                                    hsb[:, jt, :],
                                    start=(jt == 0), stop=(jt == JT - 1))
                            nc.scalar.activation(
                                o_sb[:, hi, s0:s0 + SS_CHUNK], ps[:],
                                mybir.ActivationFunctionType.Identity,
                                bias=b_col[:, hi:hi + 1], scale=scale)
                    for st in range(SS_CHUNK // 128):
                        ps = p1p.tile([128, CI], F32, name="ps_v")
                        nc.tensor.matmul(ps[:], ones_row_bf[:], bv_sb[:],
                                         start=True, stop=False)
                        for jt in range(JT):
                            nc.tensor.matmul(
                                ps[:],
                                hsb[:, jt, st * 128:(st + 1) * 128],
                                wv_sb[:, jt, :],
                                start=False, stop=(jt == JT - 1))
                        nc.scalar.copy(v_sb[:, ch * 4 + st, :], ps[:])

            # ---- phase 2: attention ----
            with (
                tc.tile_pool(name="stp", bufs=3,
                             space=bass.MemorySpace.PSUM) as stp,
                tc.tile_pool(name="ptp", bufs=4) as ptp,
                tc.tile_pool(name="accp", bufs=2,
                             space=bass.MemorySpace.PSUM) as accp,
                tc.tile_pool(name="denp", bufs=2,
                             space=bass.MemorySpace.PSUM) as denp,
                tc.tile_pool(name="bcp", bufs=1,
                             space=bass.MemorySpace.PSUM) as bcp,
                tc.tile_pool(name="normp", bufs=3) as normp,
            ):
                for b in range(B):
                    for hi in range(HPC):
                        bh = b * HPC + hi
                        for qc in range(QC_PER_B):
                            q0 = b * S + qc * SS_CHUNK
                            ctx_ps = accp.tile([128, SS_CHUNK], F32,
                                               name="ctx_ps")
                            den_ps = denp.tile([1, SS_CHUNK], F32,
                                               name="den_ps")
                            for kt in range(KT):
                                k0 = b * S + kt * 128
                                st_ps = stp.tile([128, SS_CHUNK], F32,
                                                 name="st_ps")
                                nc.tensor.matmul(
                                    st_ps[:],
                                    kT_sb[:, hi, k0:k0 + 128],
                                    qT_sb[:, hi, q0:q0 + SS_CHUNK],
                                    start=True, stop=True)
                                pt = ptp.tile([128, SS_CHUNK], BF, name="pt")
                                # q is already pre-scaled by INV_NORM in
                                # phase 1, so exp uses scale=1
                                nc.scalar.activation(
                                    pt[:], st_ps[:],
                                    mybir.ActivationFunctionType.Exp,
                                    bias=alibi_sb[:, bh, kt:kt + 1])
                                nc.tensor.matmul(
                                    ctx_ps[:],
                                    v_sb[:, (b * S) // 128 + kt,
                                         hi * 128:(hi + 1) * 128],
                                    pt[:],
                                    start=(kt == 0), stop=(kt == KT - 1))
                                nc.tensor.matmul(
                                    den_ps[:], ones_col_bf[:], pt[:],
                                    start=(kt == 0), stop=(kt == KT - 1))
                            # den [1,512] -> SBUF -> broadcast to 128
                            # partitions via K=1 matmul -> reciprocal on
                            # all 128 lanes (cheap) -> scale ctx
                            den_sb = normp.tile([1, SS_CHUNK], F32,
                                                name="den_sb")
                            nc.scalar.copy(den_sb[:], den_ps[:])
                            denb_ps = bcp.tile([128, SS_CHUNK], F32,
                                               name="denb_ps")
                            nc.tensor.matmul(denb_ps[:], ones_row_f32[:],
                                             den_sb[:], start=True,
                                             stop=True)
                            denb_sb = normp.tile([128, SS_CHUNK], F32,
                                                 name="denb_sb")
                            nc.vector.reciprocal(denb_sb[:], denb_ps[:])
                            ctxn_sb = normp.tile([128, SS_CHUNK], BF,
                                                 name="ctxn_sb")
                            nc.vector.tensor_mul(ctxn_sb[:], ctx_ps[:],
                                                 denb_sb[:])
                            nc.sync.dma_start(
                                ctxT_local[hi * 128:(hi + 1) * 128,
                                           q0:q0 + SS_CHUNK],
                                ctxn_sb[:])

            # ---- phase 3: AllGather ctx^T ----
            nc.gpsimd.collective_compute(
                "AllGather", mybir.AluOpType.bypass,
                replica_groups=[list(range(N_CORES))],
                ins=[ctxT_local[:]], outs=[ctxT_full[:]])

            if DEBUG_OUTPUTS:
                nc.sync.dma_start(qT_dbg[:],
                                  qT_sb[:].rearrange("p a b -> p (a b)"))
                nc.sync.dma_start(kT_dbg[:],
                                  kT_sb[:].rearrange("p a b -> p (a b)"))
                nc.sync.dma_start(v_dbg[:],
                                  v_sb[:].rearrange("p a b -> p (a b)"))
                nc.sync.dma_start(ctxT_dbg[:], ctxT_full[:])

            # ---- phase 4: output projection + bias + residual ----
            with (
                tc.tile_pool(name="ctile", bufs=3) as ctp,
                tc.tile_pool(name="dpsum", bufs=SS_BLOCK,
                             space=bass.MemorySpace.PSUM) as dpp,
                tc.tile_pool(name="outp", bufs=3) as outp,
            ):
                for blk in range(N_BLOCKS):
                    ss0 = blk * SS_BLOCK * 128
                    dps = [dpp.tile([128, CI], F32, name="dps")
                           for _ in range(SS_BLOCK)]
                    for sst in range(SS_BLOCK):
                        nc.tensor.matmul(dps[sst][:], ones_row_bf[:],
                                         bd_sb[:], start=True, stop=False)
                    for it in range(IT):
                        ctile = ctp.tile([128, SS_BLOCK * 128], BF,
                                         name="ctile")
                        nc.sync.dma_start(
                            ctile[:],
                            ctxT_full[it * 128:(it + 1) * 128,
                                      ss0:ss0 + SS_BLOCK * 128])
                        for sst in range(SS_BLOCK):
                            nc.tensor.matmul(
                                dps[sst][:],
                                ctile[:, sst * 128:(sst + 1) * 128],
                                wd_sb[:, it, :],
                                start=False, stop=(it == IT - 1))
                    for sst in range(SS_BLOCK):
                        r0 = ss0 + sst * 128
                        rtile = outp.tile([128, CI], F32, name="rtile")
                        nc.sync.dma_start(rtile[:], resid[r0:r0 + 128, :])
                        osb = outp.tile([128, CI], F32, name="osb")
                        nc.vector.tensor_add(osb[:], dps[sst][:], rtile[:])
                        nc.sync.dma_start(out[r0:r0 + 128, :], osb[:])

    nc.compile()
    return nc


_NC = None


def _get_nc():
    global _NC
    if _NC is None:
        _NC = _build()
    return _NC


def _prep_in_maps(hidden_states, residual, alibi, Wq, bq, Wk, bk, Wv, bv,
                  Wd, bd):
    hs = np.ascontiguousarray(np.asarray(hidden_states, np.float32)
                              .reshape(BS, H))
    hsT_bf = np.ascontiguousarray(hs.T).astype(BF16)
    resid = np.asarray(residual, np.float32).reshape(BS, H)
    alibi_r = np.asarray(alibi, np.float32).reshape(B, NH, S)
    in_maps = []
    for c in range(N_CORES):
        sl = slice(c * CI, (c + 1) * CI)
        # alibi rows ordered (b, hi) to match kernel indexing bh = b*HPC+hi
        al = np.ascontiguousarray(
            alibi_r[:, c * HPC:(c + 1) * HPC, :].reshape(B * HPC, S))
        in_maps.append({
            "hsT": hsT_bf,
            "wqT": np.ascontiguousarray(np.asarray(Wq, np.float32)[sl].T)
                     .astype(BF16),
            "wkT": np.ascontiguousarray(np.asarray(Wk, np.float32)[sl].T)
                     .astype(BF16),
            "wvT": np.ascontiguousarray(np.asarray(Wv, np.float32)[sl].T)
                     .astype(BF16),
            "wdT": np.ascontiguousarray(np.asarray(Wd, np.float32)[sl].T)
                     .astype(BF16),
            "bq": np.asarray(bq, np.float32)[sl].reshape(CI, 1),
            "bk": np.asarray(bk, np.float32)[sl].reshape(CI, 1),
            "bv": np.asarray(bv, np.float32)[sl].reshape(1, CI).astype(BF16),
            "bd": np.asarray(bd, np.float32)[sl].reshape(1, CI).astype(BF16),
            "alibi": al,
            "resid": np.ascontiguousarray(resid[:, sl]),
        })
    return in_maps


def run(trace=False, trace_cores=None, stitch_traces=False, **inputs):
    nc = _get_nc()
    in_maps = _prep_in_maps(**inputs)
    res = bass_utils.run_bass_kernel_spmd(
        nc, in_maps, core_ids=list(range(N_CORES)), trace=trace,
        trace_cores=trace_cores, stitch_traces=stitch_traces)
    full = np.empty((BS, H), np.float32)
    for c in range(N_CORES):
        full[:, c * CI:(c + 1) * CI] = res.results[c]["out"]
    return full.reshape(B, S, H), res


def kernel(**inputs):
    out, _ = run(trace=False, **inputs)
    return out


# revision 11
# speedup vs baseline: 1.0029x; 1.0029x over previous
"""Bloom attention (separated QKV) — 8-core TRN2 Bass kernel.

Distribution: tensor-parallel over heads (2 heads/core). Each core:
  1. QKV projections for its 256-row slice of Wq/Wk/Wv (q^T,k^T in [d,s]
     layout, v in [s,d] layout, all bf16 in SBUF, fp32 accumulate).
  2. Attention with transposed scores St[k,q] = k @ q^T computed in
     qq=1024 groups, exp via ScalarE (alibi as per-partition bias),
     softmax denominator via ones-matmul, ctx^T = v^T @ P in PSUM,
     normalized by broadcast 1/den.
  3. Chunked AllGather (4 chunks along the sequence) of ctx^T slices
     (bf16), overlapped with the remaining attention blocks.
  4. Output projection for its 256-column slice of Wd + bias + residual,
     per gathered chunk.
Host side: transpose/slice/cast weights + hs (layout prep only),
concatenate the 8 output column-slices.
"""
import numpy as np
import ml_dtypes

import concourse.bass as bass
import concourse.bacc as bacc
import concourse.mybir as mybir
import concourse.tile as tile
import concourse.bass_utils as bass_utils

BF16 = ml_dtypes.bfloat16
N_CORES = 8
B, S, H = 2, 2048, 2048
NH, HD = 16, 128
HPC = NH // N_CORES          # heads per core
CI = HPC * HD                # per-core slice of H (256)
BS = B * S                   # 4096
INV_NORM = 1.0 / float(np.sqrt(HD))

JT = H // 128                # 16 contraction tiles for projections
SS_CHUNK = 512               # seq chunk for projections
N_CHUNKS = BS // SS_CHUNK    # 8
KT = S // 128                # 16 key tiles per batch
IT = H // 128                # 16 contraction tiles for dense
QBLK = 1024                  # attention/AG/dense block along seq
N_BLOCKS = BS // QBLK        # 4

F32 = mybir.dt.float32
BF = mybir.dt.bfloat16

DEBUG_OUTPUTS = False


def _build():
    nc = bacc.Bacc("TRN2", target_bir_lowering=False, debug=False,
                   num_devices=N_CORES)

    hsT = nc.dram_tensor("hsT", [H, BS], BF, kind="ExternalInput").ap()
    wqT = nc.dram_tensor("wqT", [H, CI], BF, kind="ExternalInput").ap()
    wkT = nc.dram_tensor("wkT", [H, CI], BF, kind="ExternalInput").ap()
    wvT = nc.dram_tensor("wvT", [H, CI], BF, kind="ExternalInput").ap()
    wdT = nc.dram_tensor("wdT", [H, CI], BF, kind="ExternalInput").ap()
    bq = nc.dram_tensor("bq", [CI, 1], F32, kind="ExternalInput").ap()
    bk = nc.dram_tensor("bk", [CI, 1], F32, kind="ExternalInput").ap()
    bv = nc.dram_tensor("bv", [1, CI], BF, kind="ExternalInput").ap()
    bd = nc.dram_tensor("bd", [1, CI], BF, kind="ExternalInput").ap()
    alibi = nc.dram_tensor("alibi", [B * HPC, S], F32, kind="ExternalInput").ap()
    resid = nc.dram_tensor("resid", [BS, CI], F32, kind="ExternalInput").ap()
    out = nc.dram_tensor("out", [BS, CI], F32, kind="ExternalOutput").ap()

    bounce = nc.dram_tensor("bounce", [N_BLOCKS, CI, QBLK], BF,
                            kind="Internal").ap()
    gath = nc.dram_tensor("gath", [N_BLOCKS, H, QBLK], BF, kind="Internal",
                          addr_space="Shared").ap()
    if DEBUG_OUTPUTS:
        qT_dbg = nc.dram_tensor("qT_dbg", [128, HPC * BS], BF,
                                kind="ExternalOutput").ap()
        kT_dbg = nc.dram_tensor("kT_dbg", [128, HPC * BS], BF,
                                kind="ExternalOutput").ap()
        v_dbg = nc.dram_tensor("v_dbg", [128, (BS // 128) * CI], BF,
                               kind="ExternalOutput").ap()
        ctxT_dbg = nc.dram_tensor("ctxT_dbg", [H, BS], BF,
                                  kind="ExternalOutput").ap()

    with tile.TileContext(nc) as tc:
        with (
            tc.tile_pool(name="const", bufs=1) as constp,
            tc.tile_pool(name="qkv", bufs=1) as qkvp,
        ):
            # ---- phase 0: constants (phase-1 critical ones first) ----
            wq_sb = constp.tile([128, JT, CI], BF)
            wk_sb = constp.tile([128, JT, CI], BF)
            wv_sb = constp.tile([128, JT, CI], BF)
            for w_sb, w_dr in ((wq_sb, wqT), (wk_sb, wkT), (wv_sb, wvT)):
                nc.sync.dma_start(
                    w_sb[:], w_dr.rearrange("(jt p) i -> p jt i", p=128))
            bq_sb = constp.tile([128, HPC], F32)
            bk_sb = constp.tile([128, HPC], F32)
            for b_sb, b_dr in ((bq_sb, bq), (bk_sb, bk)):
                for hi in range(HPC):
                    nc.sync.dma_start(b_sb[:, hi:hi + 1],
                                      b_dr[hi * 128:(hi + 1) * 128, :])
            bv_sb = constp.tile([1, CI], BF)
            nc.sync.dma_start(bv_sb[:], bv[:])
            alibi_sb = constp.tile([128, B * HPC, KT], F32)
            nc.sync.dma_start(
                alibi_sb[:], alibi.rearrange("r (kt p) -> p r kt", p=128))
            ones_col_bf = constp.tile([128, 1], BF)    # den lhsT (K=128, M=1)
            ones_row_bf = constp.tile([1, 128], BF)    # bias lhsT (K=1, M=128)
            ones_row_f32 = constp.tile([1, 128], F32)  # bcast lhsT (K=1, M=128)
            nc.vector.memset(ones_col_bf[:], 1.0)
            nc.vector.memset(ones_row_bf[:], 1.0)
            nc.vector.memset(ones_row_f32[:], 1.0)

            # persistent per-core activations
            qT_sb = qkvp.tile([128, HPC, BS], BF)      # [d, hi, ss]
            kT_sb = qkvp.tile([128, HPC, BS], BF)
            v_sb = qkvp.tile([128, BS // 128, CI], BF)  # [ss%128, ss//128, i]

            # ---- phase 1: QKV projections ----
            hsT_r = hsT.rearrange("(jt p) s -> p jt s", p=128)
            with (
                tc.tile_pool(name="hsb", bufs=3) as hsp,
                tc.tile_pool(name="p1psum", bufs=4,
                             space=bass.MemorySpace.PSUM) as p1p,
            ):
                for ch in range(N_CHUNKS):
                    s0 = ch * SS_CHUNK
                    hsb = hsp.tile([128, JT, SS_CHUNK], BF, name="hsb")
                    nc.sync.dma_start(hsb[:], hsT_r[:, :, s0:s0 + SS_CHUNK])
                    for w_sb, b_col, o_sb, scale in (
                        (wq_sb, bq_sb, qT_sb, INV_NORM),
                        (wk_sb, bk_sb, kT_sb, 1.0),
                    ):
                        for hi in range(HPC):
                            ps = p1p.tile([128, SS_CHUNK], F32, name="ps_qk")
                            for jt in range(JT):
                                nc.tensor.matmul(
                                    ps[:],
                                    w_sb[:, jt, hi * 128:(hi + 1) * 128],
                                    hsb[:, jt, :],
                                    start=(jt == 0), stop=(jt == JT - 1))
                            nc.scalar.activation(
                                o_sb[:, hi, s0:s0 + SS_CHUNK], ps[:],
                                mybir.ActivationFunctionType.Identity,
                                bias=b_col[:, hi:hi + 1], scale=scale)
                    for st in range(SS_CHUNK // 128):
                        ps = p1p.tile([128, CI], F32, name="ps_v")
                        nc.tensor.matmul(ps[:], ones_row_bf[:], bv_sb[:],
                                         start=True, stop=False)
                        for jt in range(JT):
                            nc.tensor.matmul(
                                ps[:],
                                hsb[:, jt, st * 128:(st + 1) * 128],
                                wv_sb[:, jt, :],
                                start=False, stop=(jt == JT - 1))
                        nc.scalar.copy(v_sb[:, ch * 4 + st, :], ps[:])

            # late consts (dense phase only) — declared after phase 1 so
            # their DMAs don't delay the first projections
            wd_sb = constp.tile([128, IT, CI], BF)
            nc.sync.dma_start(
                wd_sb[:], wdT.rearrange("(jt p) i -> p jt i", p=128))
            bd_sb = constp.tile([1, CI], BF)
            nc.sync.dma_start(bd_sb[:], bd[:])

            # ---- phase 2+3: attention blocks + chunked AllGather ----
            with (
                tc.tile_pool(name="stp", bufs=2,
                             space=bass.MemorySpace.PSUM) as stp,
                tc.tile_pool(name="ptp", bufs=4) as ptp,
                tc.tile_pool(name="accp", bufs=1,
                             space=bass.MemorySpace.PSUM) as accp,
                tc.tile_pool(name="denp", bufs=1,
                             space=bass.MemorySpace.PSUM) as denp,
                tc.tile_pool(name="normp", bufs=3) as normp,
            ):
                for blk in range(N_BLOCKS):
                    b, qh = divmod(blk, N_BLOCKS // B)
                    q0 = b * S + qh * QBLK
                    for hi in range(HPC):
                        bh = b * HPC + hi
                        ctx_ps = accp.tile([128, QBLK], F32, name="ctx_ps")
                        den_ps = denp.tile([1, QBLK], F32, name="den_ps")
                        for kt in range(KT):
                            k0 = b * S + kt * 128
                            st_ps = stp.tile([128, 2, SS_CHUNK], F32,
                                             name="st_ps")
                            for half in range(2):
                                nc.tensor.matmul(
                                    st_ps[:, half, :],
                                    kT_sb[:, hi, k0:k0 + 128],
                                    qT_sb[:, hi,
                                          q0 + half * SS_CHUNK:
                                          q0 + (half + 1) * SS_CHUNK],
                                    start=True, stop=True)
                            pt = ptp.tile([128, 2, SS_CHUNK], BF, name="pt")
                            # q pre-scaled by INV_NORM in phase 1; alibi is
                            # a per-partition (key-position) bias
                            nc.scalar.activation(
                                pt[:], st_ps[:],
                                mybir.ActivationFunctionType.Exp,
                                bias=alibi_sb[:, bh, kt:kt + 1])
                            for half in range(2):
                                hs_ = slice(half * SS_CHUNK,
                                            (half + 1) * SS_CHUNK)
                                nc.tensor.matmul(
                                    ctx_ps[:, hs_],
                                    v_sb[:, (b * S) // 128 + kt,
                                         hi * 128:(hi + 1) * 128],
                                    pt[:, half, :],
                                    start=(kt == 0), stop=(kt == KT - 1))
                                nc.tensor.matmul(
                                    den_ps[:, hs_], ones_col_bf[:],
                                    pt[:, half, :],
                                    start=(kt == 0), stop=(kt == KT - 1))
                        den_sb = normp.tile([1, QBLK], F32, name="den_sb")
                        nc.scalar.copy(den_sb[:], den_ps[:])
                        denb_ps = stp.tile([128, 2, SS_CHUNK], F32,
                                           name="st_ps")
                        for half in range(2):
                            nc.tensor.matmul(
                                denb_ps[:, half, :], ones_row_f32[:],
                                den_sb[:, half * SS_CHUNK:
                                       (half + 1) * SS_CHUNK],
                                start=True, stop=True)
                        denb_sb = normp.tile([128, QBLK], F32,
                                             name="denb_sb")
                        nc.vector.reciprocal(
                            denb_sb[:],
                            denb_ps[:].rearrange("p a b -> p (a b)"))
                        ctxn_sb = normp.tile([128, QBLK], BF,
                                             name="ctxn_sb")
                        nc.vector.tensor_mul(ctxn_sb[:], ctx_ps[:],
                                             denb_sb[:])
                        nc.sync.dma_start(
                            bounce[blk, hi * 128:(hi + 1) * 128, :],
                            ctxn_sb[:])
                    nc.gpsimd.collective_compute(
                        "AllGather", mybir.AluOpType.bypass,
                        replica_groups=[list(range(N_CORES))],
                        ins=[bounce[blk]], outs=[gath[blk]])

            if DEBUG_OUTPUTS:
                nc.sync.dma_start(qT_dbg[:],
                                  qT_sb[:].rearrange("p a b -> p (a b)"))
                nc.sync.dma_start(kT_dbg[:],
                                  kT_sb[:].rearrange("p a b -> p (a b)"))
                nc.sync.dma_start(v_dbg[:],
                                  v_sb[:].rearrange("p a b -> p (a b)"))
                for blk in range(N_BLOCKS):
                    b, qh = divmod(blk, N_BLOCKS // B)
                    q0 = b * S + qh * QBLK
                    nc.sync.dma_start(ctxT_dbg[:, q0:q0 + QBLK], gath[blk])

            # ---- phase 4: output projection + bias + residual ----
            with (
                tc.tile_pool(name="ctile", bufs=6) as ctp,
                tc.tile_pool(name="dpsum", bufs=8,
                             space=bass.MemorySpace.PSUM) as dpp,
                tc.tile_pool(name="outp", bufs=3) as outp,
            ):
                for blk in range(N_BLOCKS):
                    b, qh = divmod(blk, N_BLOCKS // B)
                    q0 = b * S + qh * QBLK
                    dps = [dpp.tile([128, CI], F32, name="dps")
                           for _ in range(QBLK // 128)]
                    for sst in range(QBLK // 128):
                        nc.tensor.matmul(dps[sst][:], ones_row_bf[:],
                                         bd_sb[:], start=True, stop=False)
                    for it in range(IT):
                        ctile = ctp.tile([128, QBLK], BF, name="ctile")
                        nc.sync.dma_start(
                            ctile[:], gath[blk, it * 128:(it + 1) * 128, :])
                        for sst in range(QBLK // 128):
                            nc.tensor.matmul(
                                dps[sst][:],
                                ctile[:, sst * 128:(sst + 1) * 128],
                                wd_sb[:, it, :],
                                start=False, stop=(it == IT - 1))
                    for sst in range(QBLK // 128):
                        r0 = q0 + sst * 128
                        rtile = outp.tile([128, CI], F32, name="rtile")
                        nc.sync.dma_start(rtile[:], resid[r0:r0 + 128, :])
                        osb = outp.tile([128, CI], F32, name="osb")
                        nc.vector.tensor_add(osb[:], dps[sst][:], rtile[:])
                        nc.sync.dma_start(out[r0:r0 + 128, :], osb[:])

    nc.compile()
    return nc


_NC = None


def _get_nc():
    global _NC
    if _NC is None:
        _NC = _build()
    return _NC


def _prep_in_maps(hidden_states, residual, alibi, Wq, bq, Wk, bk, Wv, bv,
                  Wd, bd):
    hs = np.ascontiguousarray(np.asarray(hidden_states, np.float32)
                              .reshape(BS, H))
    hsT_bf = np.ascontiguousarray(hs.T).astype(BF16)
    resid = np.asarray(residual, np.float32).reshape(BS, H)
    alibi_r = np.asarray(alibi, np.float32).reshape(B, NH, S)
    in_maps = []
    for c in range(N_CORES):
        sl = slice(c * CI, (c + 1) * CI)
        # alibi rows ordered (b, hi) to match kernel indexing bh = b*HPC+hi
        al = np.ascontiguousarray(
            alibi_r[:, c * HPC:(c + 1) * HPC, :].reshape(B * HPC, S))
        in_maps.append({
            "hsT": hsT_bf,
            "wqT": np.ascontiguousarray(np.asarray(Wq, np.float32)[sl].T)
                     .astype(BF16),
            "wkT": np.ascontiguousarray(np.asarray(Wk, np.float32)[sl].T)
                     .astype(BF16),
            "wvT": np.ascontiguousarray(np.asarray(Wv, np.float32)[sl].T)
                     .astype(BF16),
            "wdT": np.ascontiguousarray(np.asarray(Wd, np.float32)[sl].T)
                     .astype(BF16),
            "bq": np.asarray(bq, np.float32)[sl].reshape(CI, 1),
            "bk": np.asarray(bk, np.float32)[sl].reshape(CI, 1),
            "bv": np.asarray(bv, np.float32)[sl].reshape(1, CI).astype(BF16),
            "bd": np.asarray(bd, np.float32)[sl].reshape(1, CI).astype(BF16),
            "alibi": al,
            "resid": np.ascontiguousarray(resid[:, sl]),
        })
    return in_maps


def run(trace=False, trace_cores=None, stitch_traces=False, **inputs):
    nc = _get_nc()
    in_maps = _prep_in_maps(**inputs)
    res = bass_utils.run_bass_kernel_spmd(
        nc, in_maps, core_ids=list(range(N_CORES)), trace=trace,
        trace_cores=trace_cores, stitch_traces=stitch_traces)
    full = np.empty((BS, H), np.float32)
    for c in range(N_CORES):
        full[:, c * CI:(c + 1) * CI] = res.results[c]["out"]
    return full.reshape(B, S, H), res


def kernel(**inputs):
    out, _ = run(trace=False, **inputs)
    return out


# revision 15
# speedup vs baseline: 1.1217x; 1.1185x over previous
"""Bloom attention (separated QKV) — 8-core TRN2 Bass kernel.

Distribution: tensor-parallel over heads (2 heads/core). Each core:
  1. QKV projections for its 256-row slice of Wq/Wk/Wv (q^T,k^T in [d,s]
     layout, v in [s,d] layout, all bf16 in SBUF, fp32 accumulate).
  2. Attention with transposed scores St[k,q] = k @ q^T computed in
     qq=1024 groups, exp via ScalarE (alibi as per-partition bias),
     softmax denominator via ones-matmul, ctx^T = v^T @ P in PSUM,
     normalized by broadcast 1/den.
  3. Chunked AllGather (4 chunks along the sequence) of ctx^T slices
     (bf16), overlapped with the remaining attention blocks.
  4. Output projection for its 256-column slice of Wd + bias + residual,
     per gathered chunk.
Host side: transpose/slice/cast weights + hs (layout prep only),
concatenate the 8 output column-slices.
"""
import numpy as np
import ml_dtypes

import concourse.bass as bass
import concourse.bacc as bacc
import concourse.mybir as mybir
import concourse.tile as tile
import concourse.bass_utils as bass_utils

BF16 = ml_dtypes.bfloat16
N_CORES = 8
B, S, H = 2, 2048, 2048
NH, HD = 16, 128
HPC = NH // N_CORES          # heads per core
CI = HPC * HD                # per-core slice of H (256)
BS = B * S                   # 4096
INV_NORM = 1.0 / float(np.sqrt(HD))

JT = H // 128                # 16 contraction tiles for projections
SS_CHUNK = 512               # seq chunk for projections
N_CHUNKS = BS // SS_CHUNK    # 8
KT = S // 128                # 16 key tiles per batch
IT = H // 128                # 16 contraction tiles for dense
QBLK = 1024                  # attention/AG/dense block along seq
N_BLOCKS = BS // QBLK        # 4

F32 = mybir.dt.float32
BF = mybir.dt.bfloat16

DEBUG_OUTPUTS = False


def _build():
    nc = bacc.Bacc("TRN2", target_bir_lowering=False, debug=False,
                   num_devices=N_CORES)

    hsT = nc.dram_tensor("hsT", [H, BS], BF, kind="ExternalInput").ap()
    wqT = nc.dram_tensor("wqT", [H, CI], BF, kind="ExternalInput").ap()
    wkT = nc.dram_tensor("wkT", [H, CI], BF, kind="ExternalInput").ap()
    wvT = nc.dram_tensor("wvT", [H, CI], BF, kind="ExternalInput").ap()
    wdT = nc.dram_tensor("wdT", [H, CI], BF, kind="ExternalInput").ap()
    bq = nc.dram_tensor("bq", [CI, 1], F32, kind="ExternalInput").ap()
    bk = nc.dram_tensor("bk", [CI, 1], F32, kind="ExternalInput").ap()
    bv = nc.dram_tensor("bv", [1, CI], BF, kind="ExternalInput").ap()
    bd = nc.dram_tensor("bd", [1, CI], BF, kind="ExternalInput").ap()
    alibi = nc.dram_tensor("alibi", [B * HPC, S], F32, kind="ExternalInput").ap()
    resid = nc.dram_tensor("resid", [BS, CI], F32, kind="ExternalInput").ap()
    out = nc.dram_tensor("out", [BS, CI], F32, kind="ExternalOutput").ap()

    bounce = nc.dram_tensor("bounce", [N_BLOCKS, CI, QBLK], BF,
                            kind="Internal").ap()
    gath = nc.dram_tensor("gath", [N_BLOCKS, H, QBLK], BF, kind="Internal",
                          addr_space="Shared").ap()
    if DEBUG_OUTPUTS:
        qT_dbg = nc.dram_tensor("qT_dbg", [128, HPC * BS], BF,
                                kind="ExternalOutput").ap()
        kT_dbg = nc.dram_tensor("kT_dbg", [128, HPC * BS], BF,
                                kind="ExternalOutput").ap()
        v_dbg = nc.dram_tensor("v_dbg", [128, (BS // 128) * CI], BF,
                               kind="ExternalOutput").ap()
        ctxT_dbg = nc.dram_tensor("ctxT_dbg", [H, BS], BF,
                                  kind="ExternalOutput").ap()

    with tile.TileContext(nc) as tc:
        with (
            tc.tile_pool(name="const", bufs=1) as constp,
            tc.tile_pool(name="qkv", bufs=1) as qkvp,
        ):
            # ---- phase 0: constants (phase-1 critical ones first) ----
            wq_sb = constp.tile([128, JT, CI], BF)
            wk_sb = constp.tile([128, JT, CI], BF)
            wv_sb = constp.tile([128, JT, CI], BF)
            for w_sb, w_dr in ((wq_sb, wqT), (wk_sb, wkT), (wv_sb, wvT)):
                nc.sync.dma_start(
                    w_sb[:], w_dr.rearrange("(jt p) i -> p jt i", p=128))
            bq_sb = constp.tile([128, HPC], F32)
            bk_sb = constp.tile([128, HPC], F32)
            for b_sb, b_dr in ((bq_sb, bq), (bk_sb, bk)):
                for hi in range(HPC):
                    nc.sync.dma_start(b_sb[:, hi:hi + 1],
                                      b_dr[hi * 128:(hi + 1) * 128, :])
            bv_sb = constp.tile([1, CI], BF)
            nc.sync.dma_start(bv_sb[:], bv[:])
            alibi_sb = constp.tile([128, B * HPC, KT], F32)
            nc.sync.dma_start(
                alibi_sb[:], alibi.rearrange("r (kt p) -> p r kt", p=128))
            ones_col_f32 = constp.tile([128, 1], F32)  # den lhsT (K=128, M=1)
            ones_row_bf = constp.tile([1, 128], BF)    # bias lhsT (K=1, M=128)
            ones_row_f32 = constp.tile([1, 128], F32)  # bcast lhsT (K=1, M=128)
            nc.vector.memset(ones_col_f32[:], 1.0)
            nc.vector.memset(ones_row_bf[:], 1.0)
            nc.vector.memset(ones_row_f32[:], 1.0)

            # persistent per-core activations
            qT_sb = qkvp.tile([128, HPC, BS], BF)      # [d, hi, ss]
            kT_sb = qkvp.tile([128, HPC, BS], BF)
            v_sb = qkvp.tile([128, BS // 128, CI], BF)  # [ss%128, ss//128, i]

            # ---- phase 1: QKV projections ----
            hsT_r = hsT.rearrange("(jt p) s -> p jt s", p=128)
            with (
                tc.tile_pool(name="hsb", bufs=3) as hsp,
                tc.tile_pool(name="p1psum", bufs=4,
                             space=bass.MemorySpace.PSUM) as p1p,
            ):
                for ch in range(N_CHUNKS):
                    s0 = ch * SS_CHUNK
                    hsb = hsp.tile([128, JT, SS_CHUNK], BF, name="hsb")
                    nc.sync.dma_start(hsb[:], hsT_r[:, :, s0:s0 + SS_CHUNK])
                    for w_sb, b_col, o_sb, scale in (
                        (wq_sb, bq_sb, qT_sb, INV_NORM),
                        (wk_sb, bk_sb, kT_sb, 1.0),
                    ):
                        for hi in range(HPC):
                            ps = p1p.tile([128, SS_CHUNK], F32, name="ps_qk")
                            for jt in range(JT):
                                nc.tensor.matmul(
                                    ps[:],
                                    w_sb[:, jt, hi * 128:(hi + 1) * 128],
                                    hsb[:, jt, :],
                                    start=(jt == 0), stop=(jt == JT - 1))
                            nc.scalar.activation(
                                o_sb[:, hi, s0:s0 + SS_CHUNK], ps[:],
                                mybir.ActivationFunctionType.Identity,
                                bias=b_col[:, hi:hi + 1], scale=scale)
                    for st in range(SS_CHUNK // 128):
                        ps = p1p.tile([128, CI], F32, name="ps_v")
                        nc.tensor.matmul(ps[:], ones_row_bf[:], bv_sb[:],
                                         start=True, stop=False)
                        for jt in range(JT):
                            nc.tensor.matmul(
                                ps[:],
                                hsb[:, jt, st * 128:(st + 1) * 128],
                                wv_sb[:, jt, :],
                                start=False, stop=(jt == JT - 1))
                        nc.scalar.copy(v_sb[:, ch * 4 + st, :], ps[:])

            # late consts (dense phase only) — declared after phase 1 so
            # their DMAs don't delay the first projections
            wd_sb = constp.tile([128, IT, CI], BF)
            nc.sync.dma_start(
                wd_sb[:], wdT.rearrange("(jt p) i -> p jt i", p=128))
            bd_sb = constp.tile([1, CI], BF)
            nc.sync.dma_start(bd_sb[:], bd[:])

            # ---- phase 2+3: attention blocks + chunked AllGather ----
            with (
                tc.tile_pool(name="stp", bufs=2,
                             space=bass.MemorySpace.PSUM) as stp,
                tc.tile_pool(name="ptp", bufs=6) as ptp,
                tc.tile_pool(name="accp", bufs=2,
                             space=bass.MemorySpace.PSUM) as accp,
                tc.tile_pool(name="normp", bufs=3) as normp,
            ):
                for blk in range(N_BLOCKS):
                    b, qh = divmod(blk, N_BLOCKS // B)
                    q0 = b * S + qh * QBLK
                    for hi in range(HPC):
                        bh = b * HPC + hi
                        ctx_ps = accp.tile([128, QBLK], F32, name="ctx_ps")
                        acc_sb = normp.tile([128, QBLK], F32, name="acc_sb")
                        for kt in range(KT):
                            k0 = b * S + kt * 128
                            st_ps = stp.tile([128, 2, SS_CHUNK], F32,
                                             name="st_ps")
                            for half in range(2):
                                nc.tensor.matmul(
                                    st_ps[:, half, :],
                                    kT_sb[:, hi, k0:k0 + 128],
                                    qT_sb[:, hi,
                                          q0 + half * SS_CHUNK:
                                          q0 + (half + 1) * SS_CHUNK],
                                    start=True, stop=True)
                            pt = ptp.tile([128, 2, SS_CHUNK], BF, name="pt")
                            # q pre-scaled by INV_NORM in phase 1; alibi is
                            # a per-partition (key-position) bias
                            nc.scalar.activation(
                                pt[:], st_ps[:],
                                mybir.ActivationFunctionType.Exp,
                                bias=alibi_sb[:, bh, kt:kt + 1])
                            pt_flat = pt[:].rearrange("p a b -> p (a b)")
                            # denominator partial sums on DVE (off PE)
                            if kt == 0:
                                nc.vector.tensor_copy(acc_sb[:], pt_flat)
                            else:
                                nc.vector.tensor_add(acc_sb[:], acc_sb[:],
                                                     pt_flat)
                            for half in range(2):
                                hs_ = slice(half * SS_CHUNK,
                                            (half + 1) * SS_CHUNK)
                                nc.tensor.matmul(
                                    ctx_ps[:, hs_],
                                    v_sb[:, (b * S) // 128 + kt,
                                         hi * 128:(hi + 1) * 128],
                                    pt[:, half, :],
                                    start=(kt == 0), stop=(kt == KT - 1))
                        # cross-partition reduce of acc -> den, then
                        # broadcast back to 128 partitions; both borrow
                        # stp slots transiently
                        den_ps = stp.tile([128, 2, SS_CHUNK], F32,
                                          name="st_ps")
                        for half in range(2):
                            nc.tensor.matmul(
                                den_ps[:1, half, :], ones_col_f32[:],
                                acc_sb[:, half * SS_CHUNK:
                                       (half + 1) * SS_CHUNK],
                                start=True, stop=True)
                        den_sb = normp.tile([1, QBLK], F32, name="den_sb")
                        nc.vector.tensor_copy(
                            den_sb[:],
                            den_ps[:1, :, :].rearrange("p a b -> p (a b)"))
                        denb_ps = stp.tile([128, 2, SS_CHUNK], F32,
                                           name="st_ps")
                        for half in range(2):
                            nc.tensor.matmul(
                                denb_ps[:, half, :], ones_row_f32[:],
                                den_sb[:, half * SS_CHUNK:
                                       (half + 1) * SS_CHUNK],
                                start=True, stop=True)
                        denb_sb = normp.tile([128, QBLK], F32,
                                             name="denb_sb")
                        nc.vector.reciprocal(
                            denb_sb[:],
                            denb_ps[:].rearrange("p a b -> p (a b)"))
                        ctxn_sb = normp.tile([128, QBLK], BF,
                                             name="ctxn_sb")
                        nc.vector.tensor_mul(ctxn_sb[:], ctx_ps[:],
                                             denb_sb[:])
                        nc.sync.dma_start(
                            bounce[blk, hi * 128:(hi + 1) * 128, :],
                            ctxn_sb[:])
                    nc.gpsimd.collective_compute(
                        "AllGather", mybir.AluOpType.bypass,
                        replica_groups=[list(range(N_CORES))],
                        ins=[bounce[blk]], outs=[gath[blk]])

            if DEBUG_OUTPUTS:
                nc.sync.dma_start(qT_dbg[:],
                                  qT_sb[:].rearrange("p a b -> p (a b)"))
                nc.sync.dma_start(kT_dbg[:],
                                  kT_sb[:].rearrange("p a b -> p (a b)"))
                nc.sync.dma_start(v_dbg[:],
                                  v_sb[:].rearrange("p a b -> p (a b)"))
                for blk in range(N_BLOCKS):
                    b, qh = divmod(blk, N_BLOCKS // B)
                    q0 = b * S + qh * QBLK
                    nc.sync.dma_start(ctxT_dbg[:, q0:q0 + QBLK], gath[blk])

            # ---- phase 4: output projection + bias + residual ----
            with (
                tc.tile_pool(name="ctile", bufs=6) as ctp,
                tc.tile_pool(name="dpsum", bufs=8,
                             space=bass.MemorySpace.PSUM) as dpp,
                tc.tile_pool(name="outp", bufs=3) as outp,
            ):
                for blk in range(N_BLOCKS):
                    b, qh = divmod(blk, N_BLOCKS // B)
                    q0 = b * S + qh * QBLK
                    dps = [dpp.tile([128, CI], F32, name="dps")
                           for _ in range(QBLK // 128)]
                    for sst in range(QBLK // 128):
                        nc.tensor.matmul(dps[sst][:], ones_row_bf[:],
                                         bd_sb[:], start=True, stop=False)
                    for it in range(IT):
                        ctile = ctp.tile([128, QBLK], BF, name="ctile")
                        nc.sync.dma_start(
                            ctile[:], gath[blk, it * 128:(it + 1) * 128, :])
                        for sst in range(QBLK // 128):
                            nc.tensor.matmul(
                                dps[sst][:],
                                ctile[:, sst * 128:(sst + 1) * 128],
                                wd_sb[:, it, :],
                                start=False, stop=(it == IT - 1))
                    for sst in range(QBLK // 128):
                        r0 = q0 + sst * 128
                        rtile = outp.tile([128, CI], F32, name="rtile")
                        nc.sync.dma_start(rtile[:], resid[r0:r0 + 128, :])
                        osb = outp.tile([128, CI], F32, name="osb")
                        nc.vector.tensor_add(osb[:], dps[sst][:], rtile[:])
                        nc.sync.dma_start(out[r0:r0 + 128, :], osb[:])

    nc.compile()
    return nc


_NC = None


def _get_nc():
    global _NC
    if _NC is None:
        _NC = _build()
    return _NC


def _prep_in_maps(hidden_states, residual, alibi, Wq, bq, Wk, bk, Wv, bv,
                  Wd, bd):
    hs = np.ascontiguousarray(np.asarray(hidden_states, np.float32)
                              .reshape(BS, H))
    hsT_bf = np.ascontiguousarray(hs.T).astype(BF16)
    resid = np.asarray(residual, np.float32).reshape(BS, H)
    alibi_r = np.asarray(alibi, np.float32).reshape(B, NH, S)
    in_maps = []
    for c in range(N_CORES):
        sl = slice(c * CI, (c + 1) * CI)
        # alibi rows ordered (b, hi) to match kernel indexing bh = b*HPC+hi
        al = np.ascontiguousarray(
            alibi_r[:, c * HPC:(c + 1) * HPC, :].reshape(B * HPC, S))
        in_maps.append({
            "hsT": hsT_bf,
            "wqT": np.ascontiguousarray(np.asarray(Wq, np.float32)[sl].T)
                     .astype(BF16),
            "wkT": np.ascontiguousarray(np.asarray(Wk, np.float32)[sl].T)
                     .astype(BF16),
            "wvT": np.ascontiguousarray(np.asarray(Wv, np.float32)[sl].T)
                     .astype(BF16),
            "wdT": np.ascontiguousarray(np.asarray(Wd, np.float32)[sl].T)
                     .astype(BF16),
            "bq": np.asarray(bq, np.float32)[sl].reshape(CI, 1),
            "bk": np.asarray(bk, np.float32)[sl].reshape(CI, 1),
            "bv": np.asarray(bv, np.float32)[sl].reshape(1, CI).astype(BF16),
            "bd": np.asarray(bd, np.float32)[sl].reshape(1, CI).astype(BF16),
            "alibi": al,
            "resid": np.ascontiguousarray(resid[:, sl]),
        })
    return in_maps


def run(trace=False, trace_cores=None, stitch_traces=False, **inputs):
    nc = _get_nc()
    in_maps = _prep_in_maps(**inputs)
    res = bass_utils.run_bass_kernel_spmd(
        nc, in_maps, core_ids=list(range(N_CORES)), trace=trace,
        trace_cores=trace_cores, stitch_traces=stitch_traces)
    full = np.empty((BS, H), np.float32)
    for c in range(N_CORES):
        full[:, c * CI:(c + 1) * CI] = res.results[c]["out"]
    return full.reshape(B, S, H), res


def kernel(**inputs):
    out, _ = run(trace=False, **inputs)
    return out


# revision 20
# speedup vs baseline: 1.2007x; 1.0704x over previous
"""Bloom attention (separated QKV) — 8-core TRN2 Bass kernel.

Distribution: tensor-parallel over heads (2 heads/core). Each core:
  1. QKV projections for its 256-row slice of Wq/Wk/Wv (q^T,k^T in [d,s]
     layout, v in [s,d] layout, all bf16 in SBUF, fp32 accumulate).
  2. Attention with transposed scores St[k,q] = k @ q^T computed in
     qq=1024 groups, exp via ScalarE (alibi as per-partition bias),
     softmax denominator via ones-matmul, ctx^T = v^T @ P in PSUM,
     normalized by broadcast 1/den.
  3. Chunked AllGather (4 chunks along the sequence) of ctx^T slices
     (bf16), overlapped with the remaining attention blocks.
  4. Output projection for its 256-column slice of Wd + bias + residual,
     per gathered chunk.
Host side: transpose/slice/cast weights + hs (layout prep only),
concatenate the 8 output column-slices.
"""
import numpy as np
import ml_dtypes

import concourse.bass as bass
import concourse.bacc as bacc
import concourse.mybir as mybir
import concourse.tile as tile
import concourse.bass_utils as bass_utils

BF16 = ml_dtypes.bfloat16
N_CORES = 8
B, S, H = 2, 2048, 2048
NH, HD = 16, 128
HPC = NH // N_CORES          # heads per core
CI = HPC * HD                # per-core slice of H (256)
BS = B * S                   # 4096
INV_NORM = 1.0 / float(np.sqrt(HD))

JT = H // 128                # 16 contraction tiles for projections
SS_CHUNK = 512               # seq chunk for projections
N_CHUNKS = BS // SS_CHUNK    # 8
KT = S // 128                # 16 key tiles per batch
IT = H // 128                # 16 contraction tiles for dense
QBLK = 1024                  # attention/AG/dense block along seq
N_BLOCKS = BS // QBLK        # 4

F32 = mybir.dt.float32
BF = mybir.dt.bfloat16

DEBUG_OUTPUTS = False


def _build():
    nc = bacc.Bacc("TRN2", target_bir_lowering=False, debug=False,
                   num_devices=N_CORES)

    hsT = nc.dram_tensor("hsT", [H, BS], BF, kind="ExternalInput").ap()
    wqT = nc.dram_tensor("wqT", [H, CI], BF, kind="ExternalInput").ap()
    wkT = nc.dram_tensor("wkT", [H, CI], BF, kind="ExternalInput").ap()
    wvT = nc.dram_tensor("wvT", [H, CI], BF, kind="ExternalInput").ap()
    wdT = nc.dram_tensor("wdT", [H, CI], BF, kind="ExternalInput").ap()
    bq = nc.dram_tensor("bq", [CI, 1], F32, kind="ExternalInput").ap()
    bk = nc.dram_tensor("bk", [CI, 1], F32, kind="ExternalInput").ap()
    bv = nc.dram_tensor("bv", [1, CI], BF, kind="ExternalInput").ap()
    bd = nc.dram_tensor("bd", [1, CI], BF, kind="ExternalInput").ap()
    alibi = nc.dram_tensor("alibi", [B * HPC, S], F32, kind="ExternalInput").ap()
    resid = nc.dram_tensor("resid", [BS, CI], F32, kind="ExternalInput").ap()
    out = nc.dram_tensor("out", [BS, CI], F32, kind="ExternalOutput").ap()

    bounce = nc.dram_tensor("bounce", [N_BLOCKS, CI, QBLK], BF,
                            kind="Internal").ap()
    gath = nc.dram_tensor("gath", [N_BLOCKS, H, QBLK], BF, kind="Internal",
                          addr_space="Shared").ap()
    if DEBUG_OUTPUTS:
        qT_dbg = nc.dram_tensor("qT_dbg", [128, HPC * BS], BF,
                                kind="ExternalOutput").ap()
        kT_dbg = nc.dram_tensor("kT_dbg", [128, HPC * BS], BF,
                                kind="ExternalOutput").ap()
        v_dbg = nc.dram_tensor("v_dbg", [128, (BS // 128) * CI], BF,
                               kind="ExternalOutput").ap()
        ctxT_dbg = nc.dram_tensor("ctxT_dbg", [H, BS], BF,
                                  kind="ExternalOutput").ap()

    with tile.TileContext(nc) as tc:
        with (
            tc.tile_pool(name="const", bufs=1) as constp,
            tc.tile_pool(name="qkv", bufs=1) as qkvp,
        ):
            # ---- phase 0: constants (phase-1 critical ones first) ----
            wq_sb = constp.tile([128, JT, CI], BF)
            wk_sb = constp.tile([128, JT, CI], BF)
            wv_sb = constp.tile([128, JT, CI], BF)
            for w_sb, w_dr in ((wq_sb, wqT), (wk_sb, wkT), (wv_sb, wvT)):
                w_r = w_dr.rearrange("(jt p) i -> p jt i", p=128)
                nc.sync.dma_start(w_sb[:, :JT // 2, :], w_r[:, :JT // 2, :])
                nc.sync.dma_start(w_sb[:, JT // 2:, :], w_r[:, JT // 2:, :])
            bq_sb = constp.tile([128, HPC], F32)
            bk_sb = constp.tile([128, HPC], F32)
            for b_sb, b_dr in ((bq_sb, bq), (bk_sb, bk)):
                for hi in range(HPC):
                    nc.sync.dma_start(b_sb[:, hi:hi + 1],
                                      b_dr[hi * 128:(hi + 1) * 128, :])
            bv_sb = constp.tile([1, CI], BF)
            nc.sync.dma_start(bv_sb[:], bv[:])
            alibi_sb = constp.tile([128, B * HPC, KT], F32)
            nc.sync.dma_start(
                alibi_sb[:], alibi.rearrange("r (kt p) -> p r kt", p=128))
            ones_col_f32 = constp.tile([128, 1], F32)  # den lhsT (K=128, M=1)
            ones_row_bf = constp.tile([1, 128], BF)    # bias lhsT (K=1, M=128)
            ones_row_f32 = constp.tile([1, 128], F32)  # bcast lhsT (K=1, M=128)
            nc.vector.memset(ones_col_f32[:], 1.0)
            nc.vector.memset(ones_row_bf[:], 1.0)
            nc.vector.memset(ones_row_f32[:], 1.0)

            # persistent per-core activations
            qT_sb = qkvp.tile([128, HPC, BS], BF)      # [d, hi, ss]
            kT_sb = qkvp.tile([128, HPC, BS], BF)
            v_sb = qkvp.tile([128, BS // 128, CI], BF)  # [ss%128, ss//128, i]

            # ---- phase 1: QKV projections ----
            hsT_r = hsT.rearrange("(jt p) s -> p jt s", p=128)
            with (
                tc.tile_pool(name="hsb", bufs=3) as hsp,
                tc.tile_pool(name="p1psum", bufs=4,
                             space=bass.MemorySpace.PSUM) as p1p,
            ):
                for ch in range(N_CHUNKS):
                    s0 = ch * SS_CHUNK
                    hsb = hsp.tile([128, JT, SS_CHUNK], BF, name="hsb")
                    # separate queue class from the weight loads so the
                    # first projection isn't serialized behind them
                    nc.gpsimd.dma_start(hsb[:], hsT_r[:, :, s0:s0 + SS_CHUNK])
                    for w_sb, b_col, o_sb, scale in (
                        (wq_sb, bq_sb, qT_sb, INV_NORM),
                        (wk_sb, bk_sb, kT_sb, 1.0),
                    ):
                        for hi in range(HPC):
                            ps = p1p.tile([128, SS_CHUNK], F32, name="ps_qk")
                            for jt in range(JT):
                                nc.tensor.matmul(
                                    ps[:],
                                    w_sb[:, jt, hi * 128:(hi + 1) * 128],
                                    hsb[:, jt, :],
                                    start=(jt == 0), stop=(jt == JT - 1))
                            nc.scalar.activation(
                                o_sb[:, hi, s0:s0 + SS_CHUNK], ps[:],
                                mybir.ActivationFunctionType.Identity,
                                bias=b_col[:, hi:hi + 1], scale=scale)
                    for st in range(SS_CHUNK // 128):
                        ps = p1p.tile([128, CI], F32, name="ps_v")
                        nc.tensor.matmul(ps[:], ones_row_bf[:], bv_sb[:],
                                         start=True, stop=False)
                        for jt in range(JT):
                            nc.tensor.matmul(
                                ps[:],
                                hsb[:, jt, st * 128:(st + 1) * 128],
                                wv_sb[:, jt, :],
                                start=False, stop=(jt == JT - 1))
                        nc.scalar.copy(v_sb[:, ch * 4 + st, :], ps[:])

            # late consts (dense phase only) — declared after phase 1 so
            # their DMAs don't delay the first projections
            wd_sb = constp.tile([128, IT, CI], BF)
            nc.sync.dma_start(
                wd_sb[:], wdT.rearrange("(jt p) i -> p jt i", p=128))
            bd_sb = constp.tile([1, CI], BF)
            nc.sync.dma_start(bd_sb[:], bd[:])

            # ---- phase 2+3: attention blocks + chunked AllGather ----
            with (
                tc.tile_pool(name="stp", bufs=2,
                             space=bass.MemorySpace.PSUM) as stp,
                tc.tile_pool(name="ptp", bufs=6) as ptp,
                tc.tile_pool(name="accp", bufs=2,
                             space=bass.MemorySpace.PSUM) as accp,
                tc.tile_pool(name="normp", bufs=3) as normp,
            ):
                for blk in range(N_BLOCKS):
                    b, qh = divmod(blk, N_BLOCKS // B)
                    q0 = b * S + qh * QBLK
                    for hi in range(HPC):
                        bh = b * HPC + hi
                        ctx_ps = accp.tile([128, QBLK], F32, name="ctx_ps")
                        acc_sb = normp.tile([128, QBLK], F32, name="acc_sb")
                        pts = []
                        for kt in range(KT):
                            k0 = b * S + kt * 128
                            st_ps = stp.tile([128, 2, SS_CHUNK], F32,
                                             name="st_ps")
                            for half in range(2):
                                nc.tensor.matmul(
                                    st_ps[:, half, :],
                                    kT_sb[:, hi, k0:k0 + 128],
                                    qT_sb[:, hi,
                                          q0 + half * SS_CHUNK:
                                          q0 + (half + 1) * SS_CHUNK],
                                    start=True, stop=True)
                            pt = ptp.tile([128, 2, SS_CHUNK], BF, name="pt")
                            # q pre-scaled by INV_NORM in phase 1; alibi is
                            # a per-partition (key-position) bias
                            nc.scalar.activation(
                                pt[:], st_ps[:],
                                mybir.ActivationFunctionType.Exp,
                                bias=alibi_sb[:, bh, kt:kt + 1])
                            pts.append(pt)
                            # denominator partial sums on DVE (off PE):
                            # bf16 pair-sum into f32, then f32 accumulate
                            if kt % 2 == 1:
                                pa = pts[kt - 1][:].rearrange(
                                    "p a b -> p (a b)")
                                pb = pt[:].rearrange("p a b -> p (a b)")
                                psum2 = normp.tile([128, QBLK], F32,
                                                   name="psum2")
                                nc.vector.tensor_add(psum2[:], pa, pb)
                                if kt == 1:
                                    nc.vector.tensor_copy(acc_sb[:],
                                                          psum2[:])
                                else:
                                    nc.vector.tensor_add(acc_sb[:],
                                                         acc_sb[:],
                                                         psum2[:])
                            for half in range(2):
                                hs_ = slice(half * SS_CHUNK,
                                            (half + 1) * SS_CHUNK)
                                nc.tensor.matmul(
                                    ctx_ps[:, hs_],
                                    v_sb[:, (b * S) // 128 + kt,
                                         hi * 128:(hi + 1) * 128],
                                    pt[:, half, :],
                                    start=(kt == 0), stop=(kt == KT - 1))
                        # cross-partition reduce of acc -> den, then
                        # broadcast back to 128 partitions; both borrow
                        # stp slots transiently
                        den_ps = stp.tile([128, 2, SS_CHUNK], F32,
                                          name="st_ps")
                        for half in range(2):
                            nc.tensor.matmul(
                                den_ps[:1, half, :], ones_col_f32[:],
                                acc_sb[:, half * SS_CHUNK:
                                       (half + 1) * SS_CHUNK],
                                start=True, stop=True)
                        den_sb = normp.tile([1, QBLK], F32, name="den_sb")
                        nc.vector.tensor_copy(
                            den_sb[:],
                            den_ps[:1, :, :].rearrange("p a b -> p (a b)"))
                        denb_ps = stp.tile([128, 2, SS_CHUNK], F32,
                                           name="st_ps")
                        for half in range(2):
                            nc.tensor.matmul(
                                denb_ps[:, half, :], ones_row_f32[:],
                                den_sb[:, half * SS_CHUNK:
                                       (half + 1) * SS_CHUNK],
                                start=True, stop=True)
                        denb_sb = normp.tile([128, QBLK], F32,
                                             name="denb_sb")
                        nc.vector.reciprocal_approx_fast(
                            denb_sb[:],
                            denb_ps[:].rearrange("p a b -> p (a b)"))
                        ctxn_sb = normp.tile([128, QBLK], BF,
                                             name="ctxn_sb")
                        nc.vector.tensor_mul(ctxn_sb[:], ctx_ps[:],
                                             denb_sb[:])
                        nc.sync.dma_start(
                            bounce[blk, hi * 128:(hi + 1) * 128, :],
                            ctxn_sb[:])
                    nc.gpsimd.collective_compute(
                        "AllGather", mybir.AluOpType.bypass,
                        replica_groups=[list(range(N_CORES))],
                        ins=[bounce[blk]], outs=[gath[blk]])

            if DEBUG_OUTPUTS:
                nc.sync.dma_start(qT_dbg[:],
                                  qT_sb[:].rearrange("p a b -> p (a b)"))
                nc.sync.dma_start(kT_dbg[:],
                                  kT_sb[:].rearrange("p a b -> p (a b)"))
                nc.sync.dma_start(v_dbg[:],
                                  v_sb[:].rearrange("p a b -> p (a b)"))
                for blk in range(N_BLOCKS):
                    b, qh = divmod(blk, N_BLOCKS // B)
                    q0 = b * S + qh * QBLK
                    nc.sync.dma_start(ctxT_dbg[:, q0:q0 + QBLK], gath[blk])

            # ---- phase 4: output projection + bias + residual ----
            with (
                tc.tile_pool(name="ctile", bufs=16) as ctp,
                tc.tile_pool(name="dpsum", bufs=8,
                             space=bass.MemorySpace.PSUM) as dpp,
                tc.tile_pool(name="outp", bufs=3) as outp,
            ):
                for blk in range(N_BLOCKS):
                    b, qh = divmod(blk, N_BLOCKS // B)
                    q0 = b * S + qh * QBLK
                    dps = [dpp.tile([128, CI], F32, name="dps")
                           for _ in range(QBLK // 128)]
                    for sst in range(QBLK // 128):
                        nc.tensor.matmul(dps[sst][:], ones_row_bf[:],
                                         bd_sb[:], start=True, stop=False)
                    for it in range(IT):
                        ctile = ctp.tile([128, QBLK], BF, name="ctile")
                        nc.sync.dma_start(
                            ctile[:], gath[blk, it * 128:(it + 1) * 128, :])
                        for sst in range(QBLK // 128):
                            nc.tensor.matmul(
                                dps[sst][:],
                                ctile[:, sst * 128:(sst + 1) * 128],
                                wd_sb[:, it, :],
                                start=False, stop=(it == IT - 1))
                    for sst in range(QBLK // 128):
                        r0 = q0 + sst * 128
                        rtile = outp.tile([128, CI], F32, name="rtile")
                        nc.sync.dma_start(rtile[:], resid[r0:r0 + 128, :])
                        osb = outp.tile([128, CI], F32, name="osb")
                        nc.vector.tensor_add(osb[:], dps[sst][:], rtile[:])
                        nc.sync.dma_start(out[r0:r0 + 128, :], osb[:])

    nc.compile()
    return nc


_NC = None


def _get_nc():
    global _NC
    if _NC is None:
        _NC = _build()
    return _NC


def _prep_in_maps(hidden_states, residual, alibi, Wq, bq, Wk, bk, Wv, bv,
                  Wd, bd):
    hs = np.ascontiguousarray(np.asarray(hidden_states, np.float32)
                              .reshape(BS, H))
    hsT_bf = np.ascontiguousarray(hs.T).astype(BF16)
    resid = np.asarray(residual, np.float32).reshape(BS, H)
    alibi_r = np.asarray(alibi, np.float32).reshape(B, NH, S)
    in_maps = []
    for c in range(N_CORES):
        sl = slice(c * CI, (c + 1) * CI)
        # alibi rows ordered (b, hi) to match kernel indexing bh = b*HPC+hi
        al = np.ascontiguousarray(
            alibi_r[:, c * HPC:(c + 1) * HPC, :].reshape(B * HPC, S))
        in_maps.append({
            "hsT": hsT_bf,
            "wqT": np.ascontiguousarray(np.asarray(Wq, np.float32)[sl].T)
                     .astype(BF16),
            "wkT": np.ascontiguousarray(np.asarray(Wk, np.float32)[sl].T)
                     .astype(BF16),
            "wvT": np.ascontiguousarray(np.asarray(Wv, np.float32)[sl].T)
                     .astype(BF16),
            "wdT": np.ascontiguousarray(np.asarray(Wd, np.float32)[sl].T)
                     .astype(BF16),
            "bq": np.asarray(bq, np.float32)[sl].reshape(CI, 1),
            "bk": np.asarray(bk, np.float32)[sl].reshape(CI, 1),
            "bv": np.asarray(bv, np.float32)[sl].reshape(1, CI).astype(BF16),
            "bd": np.asarray(bd, np.float32)[sl].reshape(1, CI).astype(BF16),
            "alibi": al,
            "resid": np.ascontiguousarray(resid[:, sl]),
        })
    return in_maps


def run(trace=False, trace_cores=None, stitch_traces=False, **inputs):
    nc = _get_nc()
    in_maps = _prep_in_maps(**inputs)
    res = bass_utils.run_bass_kernel_spmd(
        nc, in_maps, core_ids=list(range(N_CORES)), trace=trace,
        trace_cores=trace_cores, stitch_traces=stitch_traces)
    full = np.empty((BS, H), np.float32)
    for c in range(N_CORES):
        full[:, c * CI:(c + 1) * CI] = res.results[c]["out"]
    return full.reshape(B, S, H), res


def kernel(**inputs):
    out, _ = run(trace=False, **inputs)
    return out


# revision 31
# speedup vs baseline: 1.2625x; 1.0515x over previous
"""Bloom attention (separated QKV) — 8-core TRN2 Bass kernel.

Distribution: tensor-parallel over heads (2 heads/core). Each core:
  1. QKV projections for its 256-row slice of Wq/Wk/Wv (q^T,k^T in [d,s]
     layout, v in [s,d] layout, all bf16 in SBUF, fp32 accumulate).
  2. Attention with transposed scores St[k,q] = k @ q^T computed in
     qq=1024 groups, exp via ScalarE (alibi as per-partition bias),
     softmax denominator via ones-matmul, ctx^T = v^T @ P in PSUM,
     normalized by broadcast 1/den.
  3. Chunked AllGather (4 chunks along the sequence) of ctx^T slices
     (bf16), overlapped with the remaining attention blocks.
  4. Output projection for its 256-column slice of Wd + bias + residual,
     per gathered chunk.
Host side: transpose/slice/cast weights + hs (layout prep only),
concatenate the 8 output column-slices.
"""
import numpy as np
import ml_dtypes

import concourse.bass as bass
import concourse.bacc as bacc
import concourse.mybir as mybir
import concourse.tile as tile
import concourse.bass_utils as bass_utils

BF16 = ml_dtypes.bfloat16
N_CORES = 8
B, S, H = 2, 2048, 2048
NH, HD = 16, 128
HPC = NH // N_CORES          # heads per core
CI = HPC * HD                # per-core slice of H (256)
BS = B * S                   # 4096
INV_NORM = 1.0 / float(np.sqrt(HD))

JT = H // 128                # 16 contraction tiles for projections
SS_CHUNK = 512               # seq chunk for projections
N_CHUNKS = BS // SS_CHUNK    # 8
KT = S // 128                # 16 key tiles per batch
IT = H // 128                # 16 contraction tiles for dense
QBLK = 1024                  # attention/AG/dense block along seq
N_BLOCKS = BS // QBLK        # 4

F32 = mybir.dt.float32
BF = mybir.dt.bfloat16

DEBUG_OUTPUTS = False


def _build():
    nc = bacc.Bacc("TRN2", target_bir_lowering=False, debug=False,
                   num_devices=N_CORES)

    hsT = nc.dram_tensor("hsT", [H, BS], BF, kind="ExternalInput").ap()
    wqT = nc.dram_tensor("wqT", [H, CI], BF, kind="ExternalInput").ap()
    wkT = nc.dram_tensor("wkT", [H, CI], BF, kind="ExternalInput").ap()
    wvT = nc.dram_tensor("wvT", [H, CI], BF, kind="ExternalInput").ap()
    wdT = nc.dram_tensor("wdT", [H, CI], BF, kind="ExternalInput").ap()
    bq = nc.dram_tensor("bq", [CI, 1], F32, kind="ExternalInput").ap()
    bk = nc.dram_tensor("bk", [CI, 1], F32, kind="ExternalInput").ap()
    bv = nc.dram_tensor("bv", [1, CI], BF, kind="ExternalInput").ap()
    bd_f32 = nc.dram_tensor("bd", [CI, 1], F32, kind="ExternalInput").ap()
    alibi = nc.dram_tensor("alibi", [B * HPC, S], F32, kind="ExternalInput").ap()
    residT = nc.dram_tensor("residT", [CI, BS], F32, kind="ExternalInput").ap()
    outT = nc.dram_tensor("outT", [CI, BS], F32, kind="ExternalOutput").ap()

    bounce = nc.dram_tensor("bounce", [N_BLOCKS, CI, QBLK], BF,
                            kind="Internal").ap()
    gath = nc.dram_tensor("gath", [N_BLOCKS, H, QBLK], BF, kind="Internal",
                          addr_space="Shared").ap()
    if DEBUG_OUTPUTS:
        qT_dbg = nc.dram_tensor("qT_dbg", [128, HPC * BS], BF,
                                kind="ExternalOutput").ap()
        kT_dbg = nc.dram_tensor("kT_dbg", [128, HPC * BS], BF,
                                kind="ExternalOutput").ap()
        v_dbg = nc.dram_tensor("v_dbg", [128, (BS // 128) * CI], BF,
                               kind="ExternalOutput").ap()
        ctxT_dbg = nc.dram_tensor("ctxT_dbg", [H, BS], BF,
                                  kind="ExternalOutput").ap()

    with tile.TileContext(nc) as tc:
        with (
            tc.tile_pool(name="const", bufs=1) as constp,
            tc.tile_pool(name="qkv", bufs=1) as qkvp,
        ):
            # ---- phase 0: constants (phase-1 critical ones first) ----
            wq_sb = constp.tile([128, JT, CI], BF)
            wk_sb = constp.tile([128, JT, CI], BF)
            wv_sb = constp.tile([128, JT, CI], BF)
            for w_sb, w_dr in ((wq_sb, wqT), (wk_sb, wkT), (wv_sb, wvT)):
                w_r = w_dr.rearrange("(jt p) i -> p jt i", p=128)
                nc.sync.dma_start(w_sb[:, :JT // 2, :], w_r[:, :JT // 2, :])
                nc.scalar.dma_start(w_sb[:, JT // 2:, :], w_r[:, JT // 2:, :])
            bq_sb = constp.tile([128, HPC], F32)
            bk_sb = constp.tile([128, HPC], F32)
            for b_sb, b_dr in ((bq_sb, bq), (bk_sb, bk)):
                for hi in range(HPC):
                    nc.sync.dma_start(b_sb[:, hi:hi + 1],
                                      b_dr[hi * 128:(hi + 1) * 128, :])
            bv_sb = constp.tile([1, CI], BF)
            nc.sync.dma_start(bv_sb[:], bv[:])
            alibi_sb = constp.tile([128, B * HPC, KT], F32)
            nc.sync.dma_start(
                alibi_sb[:], alibi.rearrange("r (kt p) -> p r kt", p=128))
            ones_col_f32 = constp.tile([128, 1], F32)  # den lhsT (K=128, M=1)
            ones_row_bf = constp.tile([1, 128], BF)    # bias lhsT (K=1, M=128)
            ones_row_f32 = constp.tile([1, 128], F32)  # bcast lhsT (K=1, M=128)
            nc.vector.memset(ones_col_f32[:], 1.0)
            nc.vector.memset(ones_row_bf[:], 1.0)
            nc.vector.memset(ones_row_f32[:], 1.0)

            # persistent per-core activations
            qT_sb = qkvp.tile([128, HPC, BS], BF)      # [d, hi, ss]
            kT_sb = qkvp.tile([128, HPC, BS], BF)
            v_sb = qkvp.tile([128, BS // 128, CI], BF)  # [ss%128, ss//128, i]

            # ---- phase 1: QKV projections ----
            hsT_r = hsT.rearrange("(jt p) s -> p jt s", p=128)
            with (
                tc.tile_pool(name="hsb", bufs=3) as hsp,
                tc.tile_pool(name="p1psum", bufs=4,
                             space=bass.MemorySpace.PSUM) as p1p,
            ):
                for ch in range(N_CHUNKS):
                    s0 = ch * SS_CHUNK
                    hsb = hsp.tile([128, JT, SS_CHUNK], BF, name="hsb")
                    # separate queue classes from the weight loads so the
                    # first projection isn't serialized behind them
                    nc.gpsimd.dma_start(hsb[:, :JT // 2, :],
                                        hsT_r[:, :JT // 2, s0:s0 + SS_CHUNK])
                    nc.sync.dma_start(hsb[:, JT // 2:, :],
                                      hsT_r[:, JT // 2:, s0:s0 + SS_CHUNK])
                    for w_sb, b_col, o_sb, scale in (
                        (wq_sb, bq_sb, qT_sb, INV_NORM),
                        (wk_sb, bk_sb, kT_sb, 1.0),
                    ):
                        for hi in range(HPC):
                            ps = p1p.tile([128, SS_CHUNK], F32, name="ps_qk")
                            for jt in range(JT):
                                nc.tensor.matmul(
                                    ps[:],
                                    w_sb[:, jt, hi * 128:(hi + 1) * 128],
                                    hsb[:, jt, :],
                                    start=(jt == 0), stop=(jt == JT - 1))
                            nc.scalar.activation(
                                o_sb[:, hi, s0:s0 + SS_CHUNK], ps[:],
                                mybir.ActivationFunctionType.Identity,
                                bias=b_col[:, hi:hi + 1], scale=scale)
                    for st in range(SS_CHUNK // 128):
                        ps = p1p.tile([128, CI], F32, name="ps_v")
                        nc.tensor.matmul(ps[:], ones_row_bf[:], bv_sb[:],
                                         start=True, stop=False)
                        for jt in range(JT):
                            nc.tensor.matmul(
                                ps[:],
                                hsb[:, jt, st * 128:(st + 1) * 128],
                                wv_sb[:, jt, :],
                                start=False, stop=(jt == JT - 1))
                        nc.scalar.copy(v_sb[:, ch * 4 + st, :], ps[:])

            # late consts (dense phase only) — declared after phase 1 so
            # their DMAs don't delay the first projections
            wd_sb = constp.tile([128, IT, CI], BF)
            nc.sync.dma_start(
                wd_sb[:], wdT.rearrange("(jt p) i -> p jt i", p=128))
            bd_col = constp.tile([128, HPC], F32)
            for ci in range(HPC):
                nc.sync.dma_start(bd_col[:, ci:ci + 1],
                                  bd_f32[ci * 128:(ci + 1) * 128, :])

            # ---- phase 2+3: attention blocks + chunked AllGather ----
            with (
                tc.tile_pool(name="stp", bufs=2,
                             space=bass.MemorySpace.PSUM) as stp,
                tc.tile_pool(name="ptp", bufs=10) as ptp,
                tc.tile_pool(name="accp", bufs=2,
                             space=bass.MemorySpace.PSUM) as accp,
                tc.tile_pool(name="normp", bufs=3) as normp,
            ):
                for blk in range(N_BLOCKS):
                    b, qh = divmod(blk, N_BLOCKS // B)
                    q0 = b * S + qh * QBLK
                    for hi in range(HPC):
                        bh = b * HPC + hi
                        ctx_ps = accp.tile([128, QBLK], F32, name="ctx_ps")
                        acc_sb = normp.tile([128, QBLK], F32, name="acc_sb")
                        pts = []
                        for kt in range(KT):
                            k0 = b * S + kt * 128
                            st_ps = stp.tile([128, 2, SS_CHUNK], F32,
                                             name="st_ps")
                            for half in range(2):
                                nc.tensor.matmul(
                                    st_ps[:, half, :],
                                    kT_sb[:, hi, k0:k0 + 128],
                                    qT_sb[:, hi,
                                          q0 + half * SS_CHUNK:
                                          q0 + (half + 1) * SS_CHUNK],
                                    start=True, stop=True)
                            pt = ptp.tile([128, 2, SS_CHUNK], BF, name="pt")
                            # q pre-scaled by INV_NORM in phase 1; alibi is
                            # a per-partition (key-position) bias
                            nc.scalar.activation(
                                pt[:], st_ps[:],
                                mybir.ActivationFunctionType.Exp,
                                bias=alibi_sb[:, bh, kt:kt + 1])
                            pts.append(pt)
                            # denominator partial sums on DVE (off PE):
                            # bf16 pair-sum into f32, then f32 accumulate
                            if kt % 2 == 1:
                                pa = pts[kt - 1][:].rearrange(
                                    "p a b -> p (a b)")
                                pb = pt[:].rearrange("p a b -> p (a b)")
                                # bf16 pair-sum (2x DVE rate); f32 chain
                                psum2 = normp.tile([128, QBLK], BF,
                                                   name="psum2")
                                nc.vector.tensor_add(psum2[:], pa, pb)
                                if kt == 1:
                                    nc.vector.tensor_copy(acc_sb[:],
                                                          psum2[:])
                                else:
                                    nc.vector.tensor_add(acc_sb[:],
                                                         acc_sb[:],
                                                         psum2[:])
                            for half in range(2):
                                hs_ = slice(half * SS_CHUNK,
                                            (half + 1) * SS_CHUNK)
                                nc.tensor.matmul(
                                    ctx_ps[:, hs_],
                                    v_sb[:, (b * S) // 128 + kt,
                                         hi * 128:(hi + 1) * 128],
                                    pt[:, half, :],
                                    start=(kt == 0), stop=(kt == KT - 1))
                        # cross-partition reduce of acc -> den, then
                        # broadcast back to 128 partitions; both borrow
                        # stp slots transiently
                        den_ps = stp.tile([128, 2, SS_CHUNK], F32,
                                          name="st_ps")
                        for half in range(2):
                            nc.tensor.matmul(
                                den_ps[:1, half, :], ones_col_f32[:],
                                acc_sb[:, half * SS_CHUNK:
                                       (half + 1) * SS_CHUNK],
                                start=True, stop=True)
                        den_sb = normp.tile([1, QBLK], F32, name="den_sb")
                        nc.vector.tensor_copy(
                            den_sb[:],
                            den_ps[:1, :, :].rearrange("p a b -> p (a b)"))
                        denb_ps = stp.tile([128, 2, SS_CHUNK], F32,
                                           name="st_ps")
                        for half in range(2):
                            nc.tensor.matmul(
                                denb_ps[:, half, :], ones_row_f32[:],
                                den_sb[:, half * SS_CHUNK:
                                       (half + 1) * SS_CHUNK],
                                start=True, stop=True)
                        denb_sb = normp.tile([128, QBLK], F32,
                                             name="denb_sb")
                        nc.vector.reciprocal_approx_fast(
                            denb_sb[:],
                            denb_ps[:].rearrange("p a b -> p (a b)"))
                        ctxn_sb = normp.tile([128, QBLK], BF,
                                             name="ctxn_sb")
                        nc.vector.tensor_mul(ctxn_sb[:], ctx_ps[:],
                                             denb_sb[:])
                        nc.sync.dma_start(
                            bounce[blk, hi * 128:(hi + 1) * 128, :],
                            ctxn_sb[:])
                    nc.gpsimd.collective_compute(
                        "AllGather", mybir.AluOpType.bypass,
                        replica_groups=[list(range(N_CORES))],
                        ins=[bounce[blk]], outs=[gath[blk]])

            if DEBUG_OUTPUTS:
                nc.sync.dma_start(qT_dbg[:],
                                  qT_sb[:].rearrange("p a b -> p (a b)"))
                nc.sync.dma_start(kT_dbg[:],
                                  kT_sb[:].rearrange("p a b -> p (a b)"))
                nc.sync.dma_start(v_dbg[:],
                                  v_sb[:].rearrange("p a b -> p (a b)"))
                for blk in range(N_BLOCKS):
                    b, qh = divmod(blk, N_BLOCKS // B)
                    q0 = b * S + qh * QBLK
                    nc.sync.dma_start(ctxT_dbg[:, q0:q0 + QBLK], gath[blk])

            # ---- phase 4: output projection (out^T form: Wd stationary,
            # LDWEIGHTS amortized over the moving ctx^T) + bias + residual
            with (
                tc.tile_pool(name="ctile", bufs=16) as ctp,
                tc.tile_pool(name="dpsum", bufs=8,
                             space=bass.MemorySpace.PSUM) as dpp,
                tc.tile_pool(name="outp", bufs=4) as outp,
            ):
                NSC = QBLK // SS_CHUNK      # 2 seq chunks per block
                for blk in range(N_BLOCKS):
                    b, qh = divmod(blk, N_BLOCKS // B)
                    q0 = b * S + qh * QBLK
                    dps = [dpp.tile([128, SS_CHUNK], F32, name="dps")
                           for _ in range(HPC * NSC)]
                    for it in range(IT):
                        ctile = ctp.tile([128, QBLK], BF, name="ctile")
                        nc.sync.dma_start(
                            ctile[:], gath[blk, it * 128:(it + 1) * 128, :])
                        for ct in range(HPC):
                            for sc in range(NSC):
                                nc.tensor.matmul(
                                    dps[ct * NSC + sc][:],
                                    wd_sb[:, it, ct * 128:(ct + 1) * 128],
                                    ctile[:, sc * SS_CHUNK:
                                          (sc + 1) * SS_CHUNK],
                                    start=(it == 0), stop=(it == IT - 1))
                    for ct in range(HPC):
                        for sc in range(NSC):
                            c0 = ct * 128
                            s0_ = q0 + sc * SS_CHUNK
                            rtile = outp.tile([128, SS_CHUNK], F32,
                                              name="rtile")
                            nc.sync.dma_start(
                                rtile[:],
                                residT[c0:c0 + 128, s0_:s0_ + SS_CHUNK])
                            # bias is per-partition (output channel) here
                            osb = outp.tile([128, SS_CHUNK], F32,
                                            name="osb")
                            nc.scalar.activation(
                                osb[:], dps[ct * NSC + sc][:],
                                mybir.ActivationFunctionType.Identity,
                                bias=bd_col[:, ct:ct + 1])
                            osb2 = outp.tile([128, SS_CHUNK], F32,
                                             name="osb2")
                            nc.vector.tensor_add(osb2[:], osb[:], rtile[:])
                            nc.sync.dma_start(
                                outT[c0:c0 + 128, s0_:s0_ + SS_CHUNK],
                                osb2[:])

    nc.compile()
    return nc


_NC = None


def _get_nc():
    global _NC
    if _NC is None:
        _NC = _build()
    return _NC


def _prep_in_maps(hidden_states, residual, alibi, Wq, bq, Wk, bk, Wv, bv,
                  Wd, bd):
    hs = np.ascontiguousarray(np.asarray(hidden_states, np.float32)
                              .reshape(BS, H))
    hsT_bf = np.ascontiguousarray(hs.T).astype(BF16)
    resid = np.asarray(residual, np.float32).reshape(BS, H)
    alibi_r = np.asarray(alibi, np.float32).reshape(B, NH, S)
    in_maps = []
    for c in range(N_CORES):
        sl = slice(c * CI, (c + 1) * CI)
        # alibi rows ordered (b, hi) to match kernel indexing bh = b*HPC+hi
        al = np.ascontiguousarray(
            alibi_r[:, c * HPC:(c + 1) * HPC, :].reshape(B * HPC, S))
        in_maps.append({
            "hsT": hsT_bf,
            "wqT": np.ascontiguousarray(np.asarray(Wq, np.float32)[sl].T)
                     .astype(BF16),
            "wkT": np.ascontiguousarray(np.asarray(Wk, np.float32)[sl].T)
                     .astype(BF16),
            "wvT": np.ascontiguousarray(np.asarray(Wv, np.float32)[sl].T)
                     .astype(BF16),
            "wdT": np.ascontiguousarray(np.asarray(Wd, np.float32)[sl].T)
                     .astype(BF16),
            "bq": np.asarray(bq, np.float32)[sl].reshape(CI, 1),
            "bk": np.asarray(bk, np.float32)[sl].reshape(CI, 1),
            "bv": np.asarray(bv, np.float32)[sl].reshape(1, CI).astype(BF16),
            "bd": np.asarray(bd, np.float32)[sl].reshape(CI, 1),
            "alibi": al,
            "residT": np.ascontiguousarray(resid[:, sl].T),
        })
    return in_maps


def run(trace=False, trace_cores=None, stitch_traces=False, **inputs):
    nc = _get_nc()
    in_maps = _prep_in_maps(**inputs)
    res = bass_utils.run_bass_kernel_spmd(
        nc, in_maps, core_ids=list(range(N_CORES)), trace=trace,
        trace_cores=trace_cores, stitch_traces=stitch_traces)
    full = np.empty((BS, H), np.float32)
    for c in range(N_CORES):
        full[:, c * CI:(c + 1) * CI] = res.results[c]["outT"].T
    return full.reshape(B, S, H), res


def kernel(**inputs):
    out, _ = run(trace=False, **inputs)
    return out


# revision 37
# speedup vs baseline: 1.3473x; 1.0672x over previous
"""Bloom attention (separated QKV) — 8-core TRN2 Bass kernel.

Distribution: tensor-parallel over heads (2 heads/core). Each core:
  1. QKV projections for its 256-row slice of Wq/Wk/Wv (q^T,k^T in [d,s]
     layout, v in [s,d] layout, all bf16 in SBUF, fp32 accumulate).
  2. Attention with transposed scores St[k,q] = k @ q^T computed in
     qq=1024 groups, exp via ScalarE (alibi as per-partition bias),
     softmax denominator via ones-matmul, ctx^T = v^T @ P in PSUM,
     normalized by broadcast 1/den.
  3. Chunked AllGather (4 chunks along the sequence) of ctx^T slices
     (bf16), overlapped with the remaining attention blocks.
  4. Output projection for its 256-column slice of Wd + bias + residual,
     per gathered chunk.
Host side: transpose/slice/cast weights + hs (layout prep only),
concatenate the 8 output column-slices.
"""
import numpy as np
import ml_dtypes

import concourse.bass as bass
import concourse.bacc as bacc
import concourse.mybir as mybir
import concourse.tile as tile
import concourse.bass_utils as bass_utils

BF16 = ml_dtypes.bfloat16
N_CORES = 8
B, S, H = 2, 2048, 2048
NH, HD = 16, 128
HPC = NH // N_CORES          # heads per core
CI = HPC * HD                # per-core slice of H (256)
BS = B * S                   # 4096
INV_NORM = 1.0 / float(np.sqrt(HD))

JT = H // 128                # 16 contraction tiles for projections
SS_CHUNK = 512               # seq chunk for projections
N_CHUNKS = BS // SS_CHUNK    # 8
KT = S // 128                # 16 key tiles per batch
IT = H // 128                # 16 contraction tiles for dense
QBLK = 1024                  # attention/AG/dense block along seq
N_BLOCKS = BS // QBLK        # 4

F32 = mybir.dt.float32
BF = mybir.dt.bfloat16

DEBUG_OUTPUTS = False


def _build():
    nc = bacc.Bacc("TRN2", target_bir_lowering=False, debug=False,
                   num_devices=N_CORES)

    hsT = nc.dram_tensor("hsT", [H, BS], BF, kind="ExternalInput").ap()
    wqT = nc.dram_tensor("wqT", [H, CI], BF, kind="ExternalInput").ap()
    wkT = nc.dram_tensor("wkT", [H, CI], BF, kind="ExternalInput").ap()
    wvT = nc.dram_tensor("wvT", [H, CI], BF, kind="ExternalInput").ap()
    wdT = nc.dram_tensor("wdT", [H, CI], BF, kind="ExternalInput").ap()
    bq = nc.dram_tensor("bq", [CI, 1], F32, kind="ExternalInput").ap()
    bk = nc.dram_tensor("bk", [CI, 1], F32, kind="ExternalInput").ap()
    bv = nc.dram_tensor("bv", [1, CI], BF, kind="ExternalInput").ap()
    bd_f32 = nc.dram_tensor("bd", [CI, 1], F32, kind="ExternalInput").ap()
    alibi = nc.dram_tensor("alibi", [B * HPC, S], F32, kind="ExternalInput").ap()
    residT = nc.dram_tensor("residT", [CI, BS], F32, kind="ExternalInput").ap()
    outT = nc.dram_tensor("outT", [CI, BS], F32, kind="ExternalOutput").ap()

    bounce = nc.dram_tensor("bounce", [N_BLOCKS, HPC, 128, QBLK], BF,
                            kind="Internal").ap()
    # per-(block, hi) AllGather output: rows = core*128 + d
    gath = nc.dram_tensor("gath", [HPC, N_BLOCKS, N_CORES * 128, QBLK], BF,
                          kind="Internal", addr_space="Shared").ap()
    if DEBUG_OUTPUTS:
        qT_dbg = nc.dram_tensor("qT_dbg", [128, HPC * BS], BF,
                                kind="ExternalOutput").ap()
        kT_dbg = nc.dram_tensor("kT_dbg", [128, HPC * BS], BF,
                                kind="ExternalOutput").ap()
        v_dbg = nc.dram_tensor("v_dbg", [128, (BS // 128) * CI], BF,
                               kind="ExternalOutput").ap()
        ctxT_dbg = nc.dram_tensor("ctxT_dbg", [H, BS], BF,
                                  kind="ExternalOutput").ap()

    with tile.TileContext(nc) as tc:
        with (
            tc.tile_pool(name="const", bufs=1) as constp,
            tc.tile_pool(name="qkv", bufs=1) as qkvp,
        ):
            # ---- phase 0: constants (phase-1 critical ones first) ----
            wq_sb = constp.tile([128, JT, CI], BF)
            wk_sb = constp.tile([128, JT, CI], BF)
            wv_sb = constp.tile([128, JT, CI], BF)
            for w_sb, w_dr in ((wq_sb, wqT), (wk_sb, wkT), (wv_sb, wvT)):
                w_r = w_dr.rearrange("(jt p) i -> p jt i", p=128)
                nc.scalar.dma_start(w_sb[:], w_r[:])
            bq_sb = constp.tile([128, HPC], F32)
            bk_sb = constp.tile([128, HPC], F32)
            for b_sb, b_dr in ((bq_sb, bq), (bk_sb, bk)):
                for hi in range(HPC):
                    nc.scalar.dma_start(b_sb[:, hi:hi + 1],
                                        b_dr[hi * 128:(hi + 1) * 128, :])
            bv_sb = constp.tile([1, CI], BF)
            nc.scalar.dma_start(bv_sb[:], bv[:])
            alibi_sb = constp.tile([128, B * HPC, KT], F32)
            nc.scalar.dma_start(
                alibi_sb[:], alibi.rearrange("r (kt p) -> p r kt", p=128))
            ones_col_f32 = constp.tile([128, 1], F32)  # den lhsT (K=128, M=1)
            ones_row_bf = constp.tile([1, 128], BF)    # bias lhsT (K=1, M=128)
            ones_row_f32 = constp.tile([1, 128], F32)  # bcast lhsT (K=1, M=128)
            nc.vector.memset(ones_col_f32[:], 1.0)
            nc.vector.memset(ones_row_bf[:], 1.0)
            nc.vector.memset(ones_row_f32[:], 1.0)

            # persistent per-core activations
            qT_sb = qkvp.tile([128, HPC, BS], BF)      # [d, hi, ss]
            kT_sb = qkvp.tile([128, HPC, BS], BF)
            v_sb = qkvp.tile([128, BS // 128, CI], BF)  # [ss%128, ss//128, i]

            # ---- phase 1: QKV projections ----
            hsT_r = hsT.rearrange("(jt p) s -> p jt s", p=128)
            with (
                tc.tile_pool(name="hsb", bufs=3) as hsp,
                tc.tile_pool(name="p1psum", bufs=4,
                             space=bass.MemorySpace.PSUM) as p1p,
            ):
                for ch in range(N_CHUNKS):
                    s0 = ch * SS_CHUNK
                    hsb = hsp.tile([128, JT, SS_CHUNK], BF, name="hsb")
                    # separate queue classes from the weight loads so the
                    # first projection isn't serialized behind them
                    nc.gpsimd.dma_start(hsb[:, :JT // 2, :],
                                        hsT_r[:, :JT // 2, s0:s0 + SS_CHUNK])
                    nc.sync.dma_start(hsb[:, JT // 2:, :],
                                      hsT_r[:, JT // 2:, s0:s0 + SS_CHUNK])
                    for w_sb, b_col, o_sb, scale in (
                        (wq_sb, bq_sb, qT_sb, INV_NORM),
                        (wk_sb, bk_sb, kT_sb, 1.0),
                    ):
                        for hi in range(HPC):
                            ps = p1p.tile([128, SS_CHUNK], F32, name="ps_qk")
                            for jt in range(JT):
                                nc.tensor.matmul(
                                    ps[:],
                                    w_sb[:, jt, hi * 128:(hi + 1) * 128],
                                    hsb[:, jt, :],
                                    start=(jt == 0), stop=(jt == JT - 1))
                            nc.scalar.activation(
                                o_sb[:, hi, s0:s0 + SS_CHUNK], ps[:],
                                mybir.ActivationFunctionType.Identity,
                                bias=b_col[:, hi:hi + 1], scale=scale)
                    for st in range(SS_CHUNK // 128):
                        ps = p1p.tile([128, CI], F32, name="ps_v")
                        nc.tensor.matmul(ps[:], ones_row_bf[:], bv_sb[:],
                                         start=True, stop=False)
                        for jt in range(JT):
                            nc.tensor.matmul(
                                ps[:],
                                hsb[:, jt, st * 128:(st + 1) * 128],
                                wv_sb[:, jt, :],
                                start=False, stop=(jt == JT - 1))
                        nc.scalar.copy(v_sb[:, ch * 4 + st, :], ps[:])

            # late consts (dense phase only) — declared after phase 1 so
            # their DMAs don't delay the first projections
            wd_sb = constp.tile([128, IT, CI], BF)
            nc.sync.dma_start(
                wd_sb[:], wdT.rearrange("(jt p) i -> p jt i", p=128))
            bd_col = constp.tile([128, HPC], F32)
            for ci in range(HPC):
                nc.sync.dma_start(bd_col[:, ci:ci + 1],
                                  bd_f32[ci * 128:(ci + 1) * 128, :])

            # ---- phase 2+3: attention blocks + chunked AllGather ----
            with (
                tc.tile_pool(name="stp", bufs=3,
                             space=bass.MemorySpace.PSUM) as stp,
                tc.tile_pool(name="ptp", bufs=10) as ptp,
                tc.tile_pool(name="accp", bufs=1,
                             space=bass.MemorySpace.PSUM) as accp,
                tc.tile_pool(name="normp", bufs=3) as normp,
            ):
                for blk in range(N_BLOCKS):
                    b, qh = divmod(blk, N_BLOCKS // B)
                    q0 = b * S + qh * QBLK
                    for hi in range(HPC):
                        bh = b * HPC + hi
                        ctx_ps = accp.tile([128, QBLK], F32, name="ctx_ps")
                        acc_sb = normp.tile([128, QBLK], F32, name="acc_sb")
                        pts = []
                        for kt in range(KT):
                            k0 = b * S + kt * 128
                            st_ps = stp.tile([128, 2, SS_CHUNK], F32,
                                             name="st_ps")
                            for half in range(2):
                                nc.tensor.matmul(
                                    st_ps[:, half, :],
                                    kT_sb[:, hi, k0:k0 + 128],
                                    qT_sb[:, hi,
                                          q0 + half * SS_CHUNK:
                                          q0 + (half + 1) * SS_CHUNK],
                                    start=True, stop=True)
                            pt = ptp.tile([128, 2, SS_CHUNK], BF, name="pt")
                            # q pre-scaled by INV_NORM in phase 1; alibi is
                            # a per-partition (key-position) bias
                            nc.scalar.activation(
                                pt[:], st_ps[:],
                                mybir.ActivationFunctionType.Exp,
                                bias=alibi_sb[:, bh, kt:kt + 1])
                            pts.append(pt)
                            # denominator partial sums on DVE (off PE):
                            # bf16 pair-sum into f32, then f32 accumulate
                            if kt % 2 == 1:
                                pa = pts[kt - 1][:].rearrange(
                                    "p a b -> p (a b)")
                                pb = pt[:].rearrange("p a b -> p (a b)")
                                # bf16 pair-sum (2x DVE rate); f32 chain
                                psum2 = normp.tile([128, QBLK], BF,
                                                   name="psum2")
                                nc.vector.tensor_add(psum2[:], pa, pb)
                                if kt == 1:
                                    nc.vector.tensor_copy(acc_sb[:],
                                                          psum2[:])
                                else:
                                    nc.vector.tensor_add(acc_sb[:],
                                                         acc_sb[:],
                                                         psum2[:])
                            for half in range(2):
                                hs_ = slice(half * SS_CHUNK,
                                            (half + 1) * SS_CHUNK)
                                nc.tensor.matmul(
                                    ctx_ps[:, hs_],
                                    v_sb[:, (b * S) // 128 + kt,
                                         hi * 128:(hi + 1) * 128],
                                    pt[:, half, :],
                                    start=(kt == 0), stop=(kt == KT - 1))
                        # cross-partition reduce of acc -> den, then
                        # broadcast back to 128 partitions; both borrow
                        # stp slots transiently
                        den_ps = stp.tile([128, 2, SS_CHUNK], F32,
                                          name="st_ps")
                        for half in range(2):
                            nc.tensor.matmul(
                                den_ps[:1, half, :], ones_col_f32[:],
                                acc_sb[:, half * SS_CHUNK:
                                       (half + 1) * SS_CHUNK],
                                start=True, stop=True)
                        den_sb = normp.tile([1, QBLK], F32, name="den_sb")
                        nc.vector.tensor_copy(
                            den_sb[:],
                            den_ps[:1, :, :].rearrange("p a b -> p (a b)"))
                        denb_ps = stp.tile([128, 2, SS_CHUNK], F32,
                                           name="st_ps")
                        for half in range(2):
                            nc.tensor.matmul(
                                denb_ps[:, half, :], ones_row_f32[:],
                                den_sb[:, half * SS_CHUNK:
                                       (half + 1) * SS_CHUNK],
                                start=True, stop=True)
                        denb_sb = normp.tile([128, QBLK], F32,
                                             name="denb_sb")
                        nc.vector.reciprocal_approx_fast(
                            denb_sb[:],
                            denb_ps[:].rearrange("p a b -> p (a b)"))
                        ctxn_sb = normp.tile([128, QBLK], BF,
                                             name="ctxn_sb")
                        nc.vector.tensor_mul(ctxn_sb[:], ctx_ps[:],
                                             denb_sb[:])
                        nc.sync.dma_start(bounce[blk, hi], ctxn_sb[:])
                        nc.gpsimd.collective_compute(
                            "AllGather", mybir.AluOpType.bypass,
                            replica_groups=[list(range(N_CORES))],
                            ins=[bounce[blk, hi]], outs=[gath[hi, blk]])

            if DEBUG_OUTPUTS:
                nc.sync.dma_start(qT_dbg[:],
                                  qT_sb[:].rearrange("p a b -> p (a b)"))
                nc.sync.dma_start(kT_dbg[:],
                                  kT_sb[:].rearrange("p a b -> p (a b)"))
                nc.sync.dma_start(v_dbg[:],
                                  v_sb[:].rearrange("p a b -> p (a b)"))
                dbg_r = ctxT_dbg.rearrange("(c x d) s -> c x d s", x=HPC,
                                           d=128)
                for blk in range(N_BLOCKS):
                    b, qh = divmod(blk, N_BLOCKS // B)
                    q0 = b * S + qh * QBLK
                    for hi in range(HPC):
                        nc.sync.dma_start(
                            dbg_r[:, hi, :, q0:q0 + QBLK],
                            gath[hi, blk].rearrange("(c d) s -> c d s",
                                                    d=128))

            # ---- phase 4: output projection (out^T form: Wd stationary,
            # LDWEIGHTS amortized over the moving ctx^T) + bias + residual
            with (
                tc.tile_pool(name="ctile", bufs=16) as ctp,
                tc.tile_pool(name="dpsum", bufs=8,
                             space=bass.MemorySpace.PSUM) as dpp,
                tc.tile_pool(name="outp", bufs=4) as outp,
            ):
                NSC = QBLK // SS_CHUNK      # 2 seq chunks per block
                for blk in range(N_BLOCKS):
                    b, qh = divmod(blk, N_BLOCKS // B)
                    q0 = b * S + qh * QBLK
                    dps = [dpp.tile([128, SS_CHUNK], F32, name="dps")
                           for _ in range(HPC * NSC)]
                    for it in range(IT):
                        ctile = ctp.tile([128, QBLK], BF, name="ctile")
                        nc.gpsimd.dma_start(
                            ctile[:],
                            gath[it % HPC, blk,
                                 (it // HPC) * 128:(it // HPC + 1) * 128, :])
                        for ct in range(HPC):
                            for sc in range(NSC):
                                nc.tensor.matmul(
                                    dps[ct * NSC + sc][:],
                                    wd_sb[:, it, ct * 128:(ct + 1) * 128],
                                    ctile[:, sc * SS_CHUNK:
                                          (sc + 1) * SS_CHUNK],
                                    start=(it == 0), stop=(it == IT - 1))
                    for ct in range(HPC):
                        for sc in range(NSC):
                            c0 = ct * 128
                            s0_ = q0 + sc * SS_CHUNK
                            rtile = outp.tile([128, SS_CHUNK], F32,
                                              name="rtile")
                            nc.sync.dma_start(
                                rtile[:],
                                residT[c0:c0 + 128, s0_:s0_ + SS_CHUNK])
                            # bias is per-partition (output channel) here
                            osb = outp.tile([128, SS_CHUNK], F32,
                                            name="osb")
                            nc.scalar.activation(
                                osb[:], dps[ct * NSC + sc][:],
                                mybir.ActivationFunctionType.Identity,
                                bias=bd_col[:, ct:ct + 1])
                            osb2 = outp.tile([128, SS_CHUNK], F32,
                                             name="osb2")
                            nc.vector.tensor_add(osb2[:], osb[:], rtile[:])
                            nc.sync.dma_start(
                                outT[c0:c0 + 128, s0_:s0_ + SS_CHUNK],
                                osb2[:])

    nc.compile()
    return nc


_NC = None


def _get_nc():
    global _NC
    if _NC is None:
        _NC = _build()
    return _NC


def _prep_in_maps(hidden_states, residual, alibi, Wq, bq, Wk, bk, Wv, bv,
                  Wd, bd):
    hs = np.ascontiguousarray(np.asarray(hidden_states, np.float32)
                              .reshape(BS, H))
    hsT_bf = np.ascontiguousarray(hs.T).astype(BF16)
    resid = np.asarray(residual, np.float32).reshape(BS, H)
    alibi_r = np.asarray(alibi, np.float32).reshape(B, NH, S)
    in_maps = []
    for c in range(N_CORES):
        sl = slice(c * CI, (c + 1) * CI)
        # alibi rows ordered (b, hi) to match kernel indexing bh = b*HPC+hi
        al = np.ascontiguousarray(
            alibi_r[:, c * HPC:(c + 1) * HPC, :].reshape(B * HPC, S))
        in_maps.append({
            "hsT": hsT_bf,
            "wqT": np.ascontiguousarray(np.asarray(Wq, np.float32)[sl].T)
                     .astype(BF16),
            "wkT": np.ascontiguousarray(np.asarray(Wk, np.float32)[sl].T)
                     .astype(BF16),
            "wvT": np.ascontiguousarray(np.asarray(Wv, np.float32)[sl].T)
                     .astype(BF16),
            "wdT": np.ascontiguousarray(np.asarray(Wd, np.float32)[sl].T)
                     .astype(BF16),
            "bq": np.asarray(bq, np.float32)[sl].reshape(CI, 1),
            "bk": np.asarray(bk, np.float32)[sl].reshape(CI, 1),
            "bv": np.asarray(bv, np.float32)[sl].reshape(1, CI).astype(BF16),
            "bd": np.asarray(bd, np.float32)[sl].reshape(CI, 1),
            "alibi": al,
            "residT": np.ascontiguousarray(resid[:, sl].T),
        })
    return in_maps


def run(trace=False, trace_cores=None, stitch_traces=False, **inputs):
    nc = _get_nc()
    in_maps = _prep_in_maps(**inputs)
    res = bass_utils.run_bass_kernel_spmd(
        nc, in_maps, core_ids=list(range(N_CORES)), trace=trace,
        trace_cores=trace_cores, stitch_traces=stitch_traces)
    full = np.empty((BS, H), np.float32)
    for c in range(N_CORES):
        full[:, c * CI:(c + 1) * CI] = res.results[c]["outT"].T
    return full.reshape(B, S, H), res


def kernel(**inputs):
    out, _ = run(trace=False, **inputs)
    return out


# revision 41
# speedup vs baseline: 1.4044x; 1.0423x over previous
"""Bloom attention (separated QKV) — 8-core TRN2 Bass kernel.

Distribution: tensor-parallel over heads (2 heads/core). Each core:
  1. QKV projections for its 256-row slice of Wq/Wk/Wv (q^T,k^T in [d,s]
     layout, v in [s,d] layout, all bf16 in SBUF, fp32 accumulate).
  2. Attention with transposed scores St[k,q] = k @ q^T computed in
     qq=1024 groups, exp via ScalarE (alibi as per-partition bias),
     softmax denominator via ones-matmul, ctx^T = v^T @ P in PSUM,
     normalized by broadcast 1/den.
  3. Chunked AllGather (4 chunks along the sequence) of ctx^T slices
     (bf16), overlapped with the remaining attention blocks.
  4. Output projection for its 256-column slice of Wd + bias + residual,
     per gathered chunk.
Host side: transpose/slice/cast weights + hs (layout prep only),
concatenate the 8 output column-slices.
"""
import numpy as np
import ml_dtypes

import concourse.bass as bass
import concourse.bacc as bacc
import concourse.mybir as mybir
import concourse.tile as tile
import concourse.bass_utils as bass_utils

BF16 = ml_dtypes.bfloat16
N_CORES = 8
B, S, H = 2, 2048, 2048
NH, HD = 16, 128
HPC = NH // N_CORES          # heads per core
CI = HPC * HD                # per-core slice of H (256)
BS = B * S                   # 4096
INV_NORM = 1.0 / float(np.sqrt(HD))

JT = H // 128                # 16 contraction tiles for projections
SS_CHUNK = 512               # seq chunk for projections
N_CHUNKS = BS // SS_CHUNK    # 8
KT = S // 128                # 16 key tiles per batch
IT = H // 128                # 16 contraction tiles for dense
QBLK = 1024                  # attention/AG/dense block along seq
N_BLOCKS = BS // QBLK        # 4

F32 = mybir.dt.float32
BF = mybir.dt.bfloat16

DEBUG_OUTPUTS = False


def _build():
    nc = bacc.Bacc("TRN2", target_bir_lowering=False, debug=False,
                   num_devices=N_CORES)

    hsT = nc.dram_tensor("hsT", [H, BS], BF, kind="ExternalInput").ap()
    wqT = nc.dram_tensor("wqT", [H, CI], BF, kind="ExternalInput").ap()
    wkT = nc.dram_tensor("wkT", [H, CI], BF, kind="ExternalInput").ap()
    wvT = nc.dram_tensor("wvT", [H, CI], BF, kind="ExternalInput").ap()
    wdT = nc.dram_tensor("wdT", [H, CI], BF, kind="ExternalInput").ap()
    bq = nc.dram_tensor("bq", [CI, 1], F32, kind="ExternalInput").ap()
    bk = nc.dram_tensor("bk", [CI, 1], F32, kind="ExternalInput").ap()
    bv = nc.dram_tensor("bv", [1, CI], BF, kind="ExternalInput").ap()
    bd_f32 = nc.dram_tensor("bd", [CI, 1], F32, kind="ExternalInput").ap()
    alibi = nc.dram_tensor("alibi", [B * HPC, S], F32, kind="ExternalInput").ap()
    residT = nc.dram_tensor("residT", [CI, BS], F32, kind="ExternalInput").ap()
    outT = nc.dram_tensor("outT", [CI, BS], F32, kind="ExternalOutput").ap()

    bounce = nc.dram_tensor("bounce", [N_BLOCKS, HPC, 128, QBLK], BF,
                            kind="Internal").ap()
    # per-(block, hi) AllGather output: rows = core*128 + d
    gath = nc.dram_tensor("gath", [HPC, N_BLOCKS, N_CORES * 128, QBLK], BF,
                          kind="Internal", addr_space="Shared").ap()
    if DEBUG_OUTPUTS:
        qT_dbg = nc.dram_tensor("qT_dbg", [128, HPC * BS], BF,
                                kind="ExternalOutput").ap()
        kT_dbg = nc.dram_tensor("kT_dbg", [128, HPC * BS], BF,
                                kind="ExternalOutput").ap()
        v_dbg = nc.dram_tensor("v_dbg", [128, (BS // 128) * CI], BF,
                               kind="ExternalOutput").ap()
        ctxT_dbg = nc.dram_tensor("ctxT_dbg", [H, BS], BF,
                                  kind="ExternalOutput").ap()

    with tile.TileContext(nc) as tc:
        with (
            tc.tile_pool(name="const", bufs=1) as constp,
            tc.tile_pool(name="qkv", bufs=1) as qkvp,
        ):
            # ---- phase 0: constants (phase-1 critical ones first) ----
            wq_sb = constp.tile([128, JT, CI], BF)
            wk_sb = constp.tile([128, JT, CI], BF)
            wv_sb = constp.tile([128, JT, CI], BF)
            for w_sb, w_dr in ((wq_sb, wqT), (wk_sb, wkT), (wv_sb, wvT)):
                w_r = w_dr.rearrange("(jt p) i -> p jt i", p=128)
                nc.sync.dma_start(w_sb[:, :JT // 2, :], w_r[:, :JT // 2, :])
                nc.scalar.dma_start(w_sb[:, JT // 2:, :], w_r[:, JT // 2:, :])
            bq_sb = constp.tile([128, HPC], F32)
            bk_sb = constp.tile([128, HPC], F32)
            for b_sb, b_dr in ((bq_sb, bq), (bk_sb, bk)):
                for hi in range(HPC):
                    nc.scalar.dma_start(b_sb[:, hi:hi + 1],
                                        b_dr[hi * 128:(hi + 1) * 128, :])
            bv_sb = constp.tile([1, CI], BF)
            nc.scalar.dma_start(bv_sb[:], bv[:])
            alibi_sb = constp.tile([128, B * HPC, KT], F32)
            nc.scalar.dma_start(
                alibi_sb[:], alibi.rearrange("r (kt p) -> p r kt", p=128))
            ones_col_f32 = constp.tile([128, 1], F32)  # den lhsT (K=128, M=1)
            ones_row_bf = constp.tile([1, 128], BF)    # bias lhsT (K=1, M=128)
            ones_row_f32 = constp.tile([1, 128], F32)  # bcast lhsT (K=1, M=128)
            nc.vector.memset(ones_col_f32[:], 1.0)
            nc.vector.memset(ones_row_bf[:], 1.0)
            nc.vector.memset(ones_row_f32[:], 1.0)

            # persistent per-core activations
            qT_sb = qkvp.tile([128, HPC, BS], BF)      # [d, hi, ss]
            kT_sb = qkvp.tile([128, HPC, BS], BF)
            v_sb = qkvp.tile([128, BS // 128, CI], BF)  # [ss%128, ss//128, i]

            # ---- phase 1: QKV projections ----
            hsT_r = hsT.rearrange("(jt p) s -> p jt s", p=128)
            with (
                tc.tile_pool(name="hsb", bufs=3) as hsp,
                tc.tile_pool(name="p1psum", bufs=4,
                             space=bass.MemorySpace.PSUM) as p1p,
            ):
                for ch in range(N_CHUNKS):
                    s0 = ch * SS_CHUNK
                    hsb = hsp.tile([128, JT, SS_CHUNK], BF, name="hsb")
                    # gpsimd (SWDGE) queue: fast and not shared with the
                    # weight loads
                    nc.gpsimd.dma_start(hsb[:], hsT_r[:, :, s0:s0 + SS_CHUNK])
                    for w_sb, b_col, o_sb, scale in (
                        (wq_sb, bq_sb, qT_sb, INV_NORM),
                        (wk_sb, bk_sb, kT_sb, 1.0),
                    ):
                        for hi in range(HPC):
                            ps = p1p.tile([128, SS_CHUNK], F32, name="ps_qk")
                            for jt in range(JT):
                                nc.tensor.matmul(
                                    ps[:],
                                    w_sb[:, jt, hi * 128:(hi + 1) * 128],
                                    hsb[:, jt, :],
                                    start=(jt == 0), stop=(jt == JT - 1))
                            nc.scalar.activation(
                                o_sb[:, hi, s0:s0 + SS_CHUNK], ps[:],
                                mybir.ActivationFunctionType.Identity,
                                bias=b_col[:, hi:hi + 1], scale=scale)
                    for st in range(SS_CHUNK // 128):
                        ps = p1p.tile([128, CI], F32, name="ps_v")
                        nc.tensor.matmul(ps[:], ones_row_bf[:], bv_sb[:],
                                         start=True, stop=False)
                        for jt in range(JT):
                            nc.tensor.matmul(
                                ps[:],
                                hsb[:, jt, st * 128:(st + 1) * 128],
                                wv_sb[:, jt, :],
                                start=False, stop=(jt == JT - 1))
                        nc.scalar.copy(v_sb[:, ch * 4 + st, :], ps[:])

            # late consts (dense phase only) — declared after phase 1 so
            # their DMAs don't delay the first projections
            wd_sb = constp.tile([128, IT, CI], BF)
            nc.sync.dma_start(
                wd_sb[:], wdT.rearrange("(jt p) i -> p jt i", p=128))
            bd_col = constp.tile([128, HPC], F32)
            for ci in range(HPC):
                nc.sync.dma_start(bd_col[:, ci:ci + 1],
                                  bd_f32[ci * 128:(ci + 1) * 128, :])

            # ---- phase 2+3: attention blocks + chunked AllGather ----
            with (
                tc.tile_pool(name="stp", bufs=3,
                             space=bass.MemorySpace.PSUM) as stp,
                tc.tile_pool(name="ptp", bufs=10) as ptp,
                tc.tile_pool(name="accp", bufs=1,
                             space=bass.MemorySpace.PSUM) as accp,
                tc.tile_pool(name="normp", bufs=3) as normp,
            ):
                LAG = 5
                pending_tail = [None]

                def flush_tail():
                    if pending_tail[0] is not None:
                        pending_tail[0]()
                        pending_tail[0] = None

                for blk in range(N_BLOCKS):
                    b, qh = divmod(blk, N_BLOCKS // B)
                    q0 = b * S + qh * QBLK
                    for hi in range(HPC):
                        bh = b * HPC + hi
                        ctx_ps = accp.tile([128, QBLK], F32, name="ctx_ps")
                        acc_sb = normp.tile([128, QBLK], F32, name="acc_sb")
                        pts = []

                        def consume(kt, ctx_ps=ctx_ps, acc_sb=acc_sb,
                                    pts=pts, b=b, hi=hi):
                            pt = pts[kt]
                            for half in range(2):
                                hs_ = slice(half * SS_CHUNK,
                                            (half + 1) * SS_CHUNK)
                                nc.tensor.matmul(
                                    ctx_ps[:, hs_],
                                    v_sb[:, (b * S) // 128 + kt,
                                         hi * 128:(hi + 1) * 128],
                                    pt[:, half, :],
                                    start=(kt == 0), stop=(kt == KT - 1))
                            # denominator partial sums on DVE (off PE):
                            # bf16 pair-sum (2x DVE rate), f32 chain
                            if kt % 2 == 1:
                                pa = pts[kt - 1][:].rearrange(
                                    "p a b -> p (a b)")
                                pb = pt[:].rearrange("p a b -> p (a b)")
                                psum2 = normp.tile([128, QBLK], BF,
                                                   name="psum2")
                                nc.vector.tensor_add(psum2[:], pa, pb)
                                if kt == 1:
                                    nc.vector.tensor_copy(acc_sb[:],
                                                          psum2[:])
                                else:
                                    nc.vector.tensor_add(acc_sb[:],
                                                         acc_sb[:],
                                                         psum2[:])

                        for kt in range(KT):
                            k0 = b * S + kt * 128
                            st_ps = stp.tile([128, 2, SS_CHUNK], F32,
                                             name="st_ps")
                            for half in range(2):
                                nc.tensor.matmul(
                                    st_ps[:, half, :],
                                    kT_sb[:, hi, k0:k0 + 128],
                                    qT_sb[:, hi,
                                          q0 + half * SS_CHUNK:
                                          q0 + (half + 1) * SS_CHUNK],
                                    start=True, stop=True)
                            pt = ptp.tile([128, 2, SS_CHUNK], BF, name="pt")
                            # q pre-scaled by INV_NORM in phase 1; alibi is
                            # a per-partition (key-position) bias
                            nc.scalar.activation(
                                pt[:], st_ps[:],
                                mybir.ActivationFunctionType.Exp,
                                bias=alibi_sb[:, bh, kt:kt + 1])
                            pts.append(pt)
                            # previous group's normalize tail slots in
                            # behind our first few St/exp emissions
                            if kt == 2:
                                flush_tail()
                            if kt >= LAG:
                                consume(kt - LAG)
                        for kt in range(KT - LAG, KT):
                            consume(kt)
                        # cross-partition reduce of acc -> den (borrows an
                        # stp slot; acc chain finishes under the last ctx
                        # matmuls)
                        den_ps = stp.tile([128, 2, SS_CHUNK], F32,
                                          name="st_ps")
                        for half in range(2):
                            nc.tensor.matmul(
                                den_ps[:1, half, :], ones_col_f32[:],
                                acc_sb[:, half * SS_CHUNK:
                                       (half + 1) * SS_CHUNK],
                                start=True, stop=True)
                        den_sb = normp.tile([1, QBLK], F32, name="den_sb")
                        nc.vector.tensor_copy(
                            den_sb[:],
                            den_ps[:1, :, :].rearrange("p a b -> p (a b)"))

                        def tail(ctx_ps=ctx_ps, den_sb=den_sb, blk=blk,
                                 hi=hi):
                            denb_ps = stp.tile([128, 2, SS_CHUNK], F32,
                                               name="st_ps")
                            for half in range(2):
                                nc.tensor.matmul(
                                    denb_ps[:, half, :], ones_row_f32[:],
                                    den_sb[:, half * SS_CHUNK:
                                           (half + 1) * SS_CHUNK],
                                    start=True, stop=True)
                            denb_sb = normp.tile([128, QBLK], F32,
                                                 name="denb_sb")
                            nc.vector.reciprocal_approx_fast(
                                denb_sb[:],
                                denb_ps[:].rearrange("p a b -> p (a b)"))
                            ctxn_sb = normp.tile([128, QBLK], BF,
                                                 name="ctxn_sb")
                            nc.vector.tensor_mul(ctxn_sb[:], ctx_ps[:],
                                                 denb_sb[:])
                            nc.sync.dma_start(bounce[blk, hi], ctxn_sb[:])
                            nc.gpsimd.collective_compute(
                                "AllGather", mybir.AluOpType.bypass,
                                replica_groups=[list(range(N_CORES))],
                                ins=[bounce[blk, hi]],
                                outs=[gath[hi, blk]])

                        pending_tail[0] = tail
                flush_tail()

            if DEBUG_OUTPUTS:
                nc.sync.dma_start(qT_dbg[:],
                                  qT_sb[:].rearrange("p a b -> p (a b)"))
                nc.sync.dma_start(kT_dbg[:],
                                  kT_sb[:].rearrange("p a b -> p (a b)"))
                nc.sync.dma_start(v_dbg[:],
                                  v_sb[:].rearrange("p a b -> p (a b)"))
                dbg_r = ctxT_dbg.rearrange("(c x d) s -> c x d s", x=HPC,
                                           d=128)
                for blk in range(N_BLOCKS):
                    b, qh = divmod(blk, N_BLOCKS // B)
                    q0 = b * S + qh * QBLK
                    for hi in range(HPC):
                        nc.sync.dma_start(
                            dbg_r[:, hi, :, q0:q0 + QBLK],
                            gath[hi, blk].rearrange("(c d) s -> c d s",
                                                    d=128))

            # ---- phase 4: output projection (out^T form: Wd stationary,
            # LDWEIGHTS amortized over the moving ctx^T) + bias + residual
            with (
                tc.tile_pool(name="ctile", bufs=16) as ctp,
                tc.tile_pool(name="dpsum", bufs=8,
                             space=bass.MemorySpace.PSUM) as dpp,
                tc.tile_pool(name="outp", bufs=4) as outp,
            ):
                NSC = QBLK // SS_CHUNK      # 2 seq chunks per block
                for blk in range(N_BLOCKS):
                    b, qh = divmod(blk, N_BLOCKS // B)
                    q0 = b * S + qh * QBLK
                    dps = [dpp.tile([128, SS_CHUNK], F32, name="dps")
                           for _ in range(HPC * NSC)]
                    # hi=0 rows (even it) first: their AllGather chunk
                    # lands one attention group earlier than hi=1's
                    it_order = [*range(0, IT, 2), *range(1, IT, 2)]
                    for j, it in enumerate(it_order):
                        ctile = ctp.tile([128, QBLK], BF, name="ctile")
                        nc.gpsimd.dma_start(
                            ctile[:],
                            gath[it % HPC, blk,
                                 (it // HPC) * 128:(it // HPC + 1) * 128, :])
                        for ct in range(HPC):
                            for sc in range(NSC):
                                nc.tensor.matmul(
                                    dps[ct * NSC + sc][:],
                                    wd_sb[:, it, ct * 128:(ct + 1) * 128],
                                    ctile[:, sc * SS_CHUNK:
                                          (sc + 1) * SS_CHUNK],
                                    start=(j == 0), stop=(j == IT - 1))
                    for ct in range(HPC):
                        for sc in range(NSC):
                            c0 = ct * 128
                            s0_ = q0 + sc * SS_CHUNK
                            rtile = outp.tile([128, SS_CHUNK], F32,
                                              name="rtile")
                            nc.sync.dma_start(
                                rtile[:],
                                residT[c0:c0 + 128, s0_:s0_ + SS_CHUNK])
                            # bias is per-partition (output channel) here
                            osb = outp.tile([128, SS_CHUNK], F32,
                                            name="osb")
                            nc.scalar.activation(
                                osb[:], dps[ct * NSC + sc][:],
                                mybir.ActivationFunctionType.Identity,
                                bias=bd_col[:, ct:ct + 1])
                            osb2 = outp.tile([128, SS_CHUNK], F32,
                                             name="osb2")
                            nc.vector.tensor_add(osb2[:], osb[:], rtile[:])
                            nc.sync.dma_start(
                                outT[c0:c0 + 128, s0_:s0_ + SS_CHUNK],
                                osb2[:])

    nc.compile()
    return nc


_NC = None


def _get_nc():
    global _NC
    if _NC is None:
        _NC = _build()
    return _NC


def _prep_in_maps(hidden_states, residual, alibi, Wq, bq, Wk, bk, Wv, bv,
                  Wd, bd):
    hs = np.ascontiguousarray(np.asarray(hidden_states, np.float32)
                              .reshape(BS, H))
    hsT_bf = np.ascontiguousarray(hs.T).astype(BF16)
    resid = np.asarray(residual, np.float32).reshape(BS, H)
    alibi_r = np.asarray(alibi, np.float32).reshape(B, NH, S)
    in_maps = []
    for c in range(N_CORES):
        sl = slice(c * CI, (c + 1) * CI)
        # alibi rows ordered (b, hi) to match kernel indexing bh = b*HPC+hi
        al = np.ascontiguousarray(
            alibi_r[:, c * HPC:(c + 1) * HPC, :].reshape(B * HPC, S))
        in_maps.append({
            "hsT": hsT_bf,
            "wqT": np.ascontiguousarray(np.asarray(Wq, np.float32)[sl].T)
                     .astype(BF16),
            "wkT": np.ascontiguousarray(np.asarray(Wk, np.float32)[sl].T)
                     .astype(BF16),
            "wvT": np.ascontiguousarray(np.asarray(Wv, np.float32)[sl].T)
                     .astype(BF16),
            "wdT": np.ascontiguousarray(np.asarray(Wd, np.float32)[sl].T)
                     .astype(BF16),
            "bq": np.asarray(bq, np.float32)[sl].reshape(CI, 1),
            "bk": np.asarray(bk, np.float32)[sl].reshape(CI, 1),
            "bv": np.asarray(bv, np.float32)[sl].reshape(1, CI).astype(BF16),
            "bd": np.asarray(bd, np.float32)[sl].reshape(CI, 1),
            "alibi": al,
            "residT": np.ascontiguousarray(resid[:, sl].T),
        })
    return in_maps


def run(trace=False, trace_cores=None, stitch_traces=False, **inputs):
    nc = _get_nc()
    in_maps = _prep_in_maps(**inputs)
    res = bass_utils.run_bass_kernel_spmd(
        nc, in_maps, core_ids=list(range(N_CORES)), trace=trace,
        trace_cores=trace_cores, stitch_traces=stitch_traces)
    full = np.empty((BS, H), np.float32)
    for c in range(N_CORES):
        full[:, c * CI:(c + 1) * CI] = res.results[c]["outT"].T
    return full.reshape(B, S, H), res


def kernel(**inputs):
    out, _ = run(trace=False, **inputs)
    return out
